# revision 14
# baseline (speedup 1.0000x reference)
"""Trainium2 Bass kernel for nn_AdaptiveGraphAttention (B=8,S=128,D=256,H=8).

Data-parallel: 1 sample per NeuronCore (8 cores). Per-core program:
  nv-MLP (+LayerNorm+ReLU) -> nvf ; tok=[cls;nvf] ; Q/K/V projections ;
  masked 8-head attention ; output projection.
Dead code skipped: the edge-MLP result is deleted in the reference, and
gt/aw/adj only feed a `new_adj == 0` mask which is structural (diag, col 0,
and [0,0]) because sigmoid products cannot underflow to exactly 0 for
randn-scale inputs (|logit| << 103).

Layouts: activations feature-major [D,S] where possible (host pre-transposes
inputs); scores computed in both orientations via swapped matmuls so the
attn.T @ v contraction needs no on-device transposes. Softmax uses
unnormalized exp (scores are O(1), no overflow) with multiplicative masks so
all 8 heads batch into single DVE ops.
"""

import sys

sys.path.insert(0, "/opt/trn_rl_repo")

import numpy as np
import ml_dtypes

import concourse.bass as bass
from concourse import bacc
import concourse.mybir as mybir
from concourse.bass_utils import run_bass_kernel_spmd
from concourse.tile import TileContext, ScopedClock
from concourse.masks import make_identity

BF16 = ml_dtypes.bfloat16
F32 = mybir.dt.float32
BF = mybir.dt.bfloat16
I32 = mybir.dt.int32

B, S, D = 8, 128, 256
H, HD = 8, 32
SN = S + 1  # 129
NCORES = 8

# ---------------------------------------------------------------------------
# Patch: this walrus build only supports ONE sync-wait on CTRL instructions;
# Tile's kernel-tail drain carries several. Split them across extra drains.
_PATCHED = False


def _patch_tile_drain():
    global _PATCHED
    if _PATCHED:
        return
    _PATCHED = True

    def _drain_and_barrier(self, tick_clock, wait_clock):
        nc = self.nc
        probe = nc.sync.drain()
        wait_clock.add_sem_waits(
            probe.ins, ScopedClock({None: tick_clock.global_clock})
        )
        si = probe.ins.sync_info
        waits = list(si.on_wait)
        probe.ins.sync_info = mybir.SyncInfo(
            on_wait=waits[:1], on_update=list(si.on_update)
        )
        for k in range(1, len(waits)):
            extra = nc.sync.drain()
            extra.ins.sync_info = mybir.SyncInfo(on_wait=waits[k : k + 1], on_update=[])
        nc.all_engine_barrier()
        popped = nc._tile_sem_poison_stack.pop()
        assert popped is self._sem_poison
        nc.clear_and_free_semaphores(list(self.sems.allocated().values()))
        nc.all_engine_barrier()

    TileContext._drain_and_barrier = _drain_and_barrier


# ---------------------------------------------------------------------------
# W_all row-chunk indices (each chunk = 128 rows of the stacked weight matrix)
O_SFW1 = 0  # 4 chunks   [512,256]
O_SFW2 = 4  # 2 chunks
O_WQ = 6  # 2 chunks (pre-scaled by 1/sqrt(HD))
O_WK = 8
O_WV = 10
O_WO = 12
N_WCHUNK = 14
# brows rows
R_SFB1, R_SFB2, R_BQ, R_BK, R_BV, R_BO = range(6)

_NC_CACHE = None


def _build_nc():
    global _NC_CACHE
    if _NC_CACHE is not None:
        return _NC_CACHE
    nc = bacc.Bacc()

    nvT_d = nc.declare_dram_parameter("nvT", [2 * D, S], BF, isOutput=False)
    clsT_d = nc.declare_dram_parameter("clsT", [D, 1], BF, isOutput=False)
    wall_d = nc.declare_dram_parameter("Wall", [N_WCHUNK * 128, D], BF, isOutput=False)
    brows_d = nc.declare_dram_parameter("brows", [6, D], BF, isOutput=False)
    gb_d = nc.declare_dram_parameter("gb", [2, D], F32, isOutput=False)
    attn_d = nc.declare_dram_parameter("attn", [H, SN, SN], F32, isOutput=True)
    outT_d = nc.declare_dram_parameter("outT", [D, SN], F32, isOutput=True)

    with TileContext(nc) as tc:
        with (
            tc.tile_pool(name="const", bufs=1) as constp,
            tc.tile_pool(name="acts", bufs=1) as acts,
            tc.tile_pool(name="tmp", bufs=2) as tmp,
            tc.tile_pool(name="psA", bufs=2, space="PSUM") as psA,
            tc.tile_pool(name="psB", bufs=4, space="PSUM") as psB,
            tc.tile_pool(name="psC", bufs=2, space="PSUM") as psC,
        ):
            # ---- constants (no DMA needed) ----
            ones_bf = constp.tile([1, D], BF)
            nc.vector.memset(ones_bf, 1.0)
            ident_bf = constp.tile([128, 128], BF)
            make_identity(nc, ident_bf)
            # forward mask (multiplicative): [i, h, j] zero at j==0 and j==i
            m8 = constp.tile([128, H, SN], F32)
            nc.gpsimd.memset(m8, 1.0)
            nc.gpsimd.affine_select(
                out=m8, in_=m8, compare_op=mybir.AluOpType.not_equal,
                fill=0.0, base=0, pattern=[[0, H], [-1, SN]], channel_multiplier=1,
            )  # zero where i - j == 0
            nc.gpsimd.affine_select(
                out=m8, in_=m8, compare_op=mybir.AluOpType.not_equal,
                fill=0.0, base=0, pattern=[[0, H], [1, SN]], channel_multiplier=0,
            )  # zero where j == 0
            # row-128 mask [1, h, j]: zero at j==0 and j==128
            m1 = constp.tile([1, H, SN], F32)
            nc.gpsimd.memset(m1, 1.0)
            nc.gpsimd.memset(m1[:, :, 0:1], 0.0)
            nc.gpsimd.memset(m1[:, :, S : S + 1], 0.0)

            # ---- input DMAs ----
            nvT = constp.tile([128, 4, S], BF)
            nc.sync.dma_start(nvT, nvT_d.rearrange("(o p) s -> p o s", p=128))
            wsb = constp.tile([128, N_WCHUNK, D], BF)
            nc.sync.dma_start(wsb, wall_d.rearrange("(o p) n -> p o n", p=128))
            brows = constp.tile([1, 6, D], BF)
            nc.sync.dma_start(brows, brows_d.rearrange("(u r) n -> u r n", u=1))
            g_sb = constp.tile([128, D], F32)
            nc.sync.dma_start(g_sb, gb_d[0:1, :].to_broadcast([128, D]))
            beta_sb = constp.tile([128, D], F32)
            nc.sync.dma_start(beta_sb, gb_d[1:2, :].to_broadcast([128, D]))
            tokT = acts.tile([128, 2, SN], BF)
            nc.sync.dma_start(
                tokT[:, :, 0:1], clsT_d.rearrange("(o p) u -> p o u", p=128)
            )

            # ---- 1. h_pre = nv @ sfW1 + sfb1  (token-major [i, d']) ----
            ps_h = psA.tile([128, D], F32, tag="psA")
            for k in range(4):
                nc.tensor.matmul(
                    ps_h, nvT[:, k, :], wsb[:, O_SFW1 + k, :],
                    start=(k == 0), stop=False,
                )
            nc.tensor.matmul(
                ps_h, ones_bf[0:1, 0:S], brows[0:1, R_SFB1, :],
                start=False, stop=True,
            )

            # ---- 2. LayerNorm + ReLU -> h_bf ----
            h_sb = tmp.tile([128, D], F32, tag="hsb")
            nc.vector.tensor_copy(h_sb, ps_h)
            sq_scratch = tmp.tile([128, D], F32, tag="sq")
            nc.vector.tensor_tensor(sq_scratch, h_sb, h_sb, mybir.AluOpType.mult)
            ex2r = tmp.tile([128, 1], F32, tag="s1")
            nc.vector.reduce_sum(ex2r, sq_scratch, axis=mybir.AxisListType.X)
            r1 = tmp.tile([128, 1], F32, tag="s2")
            nc.vector.reduce_sum(r1, h_sb, axis=mybir.AxisListType.X)
            mean = tmp.tile([128, 1], F32, tag="s3")
            nc.vector.tensor_scalar_mul(mean, r1, 1.0 / D)
            m2e = tmp.tile([128, 1], F32, tag="s4")
            nc.vector.tensor_tensor(m2e, mean, mean, mybir.AluOpType.mult)
            nc.vector.tensor_scalar(
                m2e, m2e, 1e-5, None, mybir.AluOpType.subtract
            )  # mean^2 - eps
            vpe = tmp.tile([128, 1], F32, tag="s5")
            nc.vector.tensor_scalar(
                vpe, ex2r, 1.0 / D, m2e, mybir.AluOpType.mult, mybir.AluOpType.subtract
            )  # var + eps
            # Newton rsqrt (no ACT Sqrt -> keeps ScalarE on the Exp table only)
            magic = tmp.tile([128, 1], I32, tag="s6")
            nc.vector.memset(magic, 0x5F3759DF)
            yi = tmp.tile([128, 1], I32, tag="s7")
            nc.vector.tensor_scalar(
                yi, vpe.bitcast(I32), 1, None, mybir.AluOpType.logical_shift_right
            )
            nc.vector.tensor_tensor(yi, magic, yi, mybir.AluOpType.subtract)
            y = yi.bitcast(F32)
            t1 = tmp.tile([128, 1], F32, tag="s8")
            for _ in range(3):
                nc.vector.tensor_tensor(t1, y, y, mybir.AluOpType.mult)
                nc.vector.tensor_tensor(t1, t1, vpe, mybir.AluOpType.mult)
                nc.vector.tensor_scalar(
                    t1, t1, -0.5, 1.5, mybir.AluOpType.mult, mybir.AluOpType.add
                )
                nc.vector.tensor_tensor(y, y, t1, mybir.AluOpType.mult)
            xn = tmp.tile([128, D], F32, tag="xn")
            nc.vector.tensor_scalar(
                xn, h_sb, mean, y, mybir.AluOpType.subtract, mybir.AluOpType.mult
            )
            nc.vector.tensor_tensor(xn, xn, g_sb, mybir.AluOpType.mult)
            nc.vector.tensor_tensor(xn, xn, beta_sb, mybir.AluOpType.add)
            h_bf = acts.tile([128, D], BF)
            nc.vector.tensor_scalar_max(h_bf, xn, 0.0)

            # ---- 3. hT via PE transpose ----
            hT = acts.tile([128, 2, S], BF)
            for c in range(2):
                ps_t = psB.tile([128, 256], BF, tag="psB")
                nc.tensor.transpose(
                    ps_t[:, 0:128], h_bf[:, c * 128 : (c + 1) * 128], ident_bf
                )
                nc.vector.tensor_copy(hT[:, c, :], ps_t[:, 0:128])

            # ---- 4. nvfT -> tokT[:, :, 1:]  (feature-major) ----
            for c in range(2):
                ps_n = psB.tile([128, 129], F32, tag="psB")
                for k in range(2):
                    nc.tensor.matmul(
                        ps_n[:, 0:S],
                        wsb[:, O_SFW2 + k, c * 128 : (c + 1) * 128],
                        hT[:, k, :],
                        start=(k == 0), stop=False,
                    )
                nc.tensor.matmul(
                    ps_n[:, 0:S],
                    brows[0:1, R_SFB2, c * 128 : (c + 1) * 128],
                    ones_bf[0:1, 0:S],
                    start=False, stop=True,
                )
                nc.vector.tensor_copy(tokT[:, c, 1:SN], ps_n[:, 0:S])

            # ---- 5. qT / kT (feature-major, head-split: [32, h, i]) ----
            qT = acts.tile([HD, H, SN], BF)
            kT = acts.tile([HD, H, SN], BF)
            for dst, oW, rB in ((qT, O_WQ, R_BQ), (kT, O_WK, R_BK)):
                for c in range(2):
                    ps_p = psB.tile([128, 129], F32, tag="psB")
                    for k in range(2):
                        nc.tensor.matmul(
                            ps_p[:, 0:SN],
                            wsb[:, oW + k, c * 128 : (c + 1) * 128],
                            tokT[:, k, :],
                            start=(k == 0), stop=False,
                        )
                    nc.tensor.matmul(
                        ps_p[:, 0:SN],
                        brows[0:1, rB, c * 128 : (c + 1) * 128],
                        ones_bf[0:1, 0:SN],
                        start=False, stop=True,
                    )
                    for hl in range(4):
                        nc.vector.tensor_copy(
                            dst[:, c * 4 + hl, :],
                            ps_p[hl * HD : (hl + 1) * HD, 0:SN],
                        )

            # ---- 6. v (token-major [j, d'], tokens 1..128 only) ----
            ps_v = psA.tile([128, D], F32, tag="psA")
            for k in range(2):
                nc.tensor.matmul(
                    ps_v, tokT[:, k, 1:SN], wsb[:, O_WV + k, :],
                    start=(k == 0), stop=False,
                )
            nc.tensor.matmul(
                ps_v, ones_bf[0:1, 0:S], brows[0:1, R_BV, :],
                start=False, stop=True,
            )
            v_bf = acts.tile([128, D], BF)
            nc.vector.tensor_copy(v_bf, ps_v)

            # ---- 7/8. per-head scores: forward exp(P) and reverse exp(u) ----
            p_all = acts.tile([128, H, SN], F32)
            p1_all = acts.tile([1, H, SN], F32)
            u_all = acts.tile([128, H, SN], BF)
            for h in range(H):
                qs = qT[:, h, :]
                ks = kT[:, h, :]
                # forward chunk0: scores[i=0..127, j]
                ps_s = psB.tile([128, 129], F32, tag="psB")
                nc.tensor.matmul(
                    ps_s[:, 0:SN], qs[:, 0:S], ks, start=True, stop=True
                )
                nc.scalar.activation(
                    p_all[:, h, :], ps_s[:, 0:SN], mybir.ActivationFunctionType.Exp
                )
                # forward chunk1: scores[i=128, j]
                ps_s1 = psC.tile([1, 256], F32, tag="psC")
                nc.tensor.matmul(
                    ps_s1[0:1, 0:SN], qs[:, S:SN], ks, start=True, stop=True
                )
                nc.scalar.activation(
                    p1_all[:, h, :],
                    ps_s1[0:1, 0:SN],
                    mybir.ActivationFunctionType.Exp,
                )
                # reverse: u[j'=j-1, i] = exp(scores[i, j])
                ps_r = psB.tile([128, 129], F32, tag="psB")
                nc.tensor.matmul(
                    ps_r[:, 0:SN], ks[:, 1:SN], qs, start=True, stop=True
                )
                nc.scalar.activation(
                    u_all[:, h, :], ps_r[:, 0:SN], mybir.ActivationFunctionType.Exp
                )
            # mask u: zero where i == j' + 1 (self-attention)
            nc.gpsimd.affine_select(
                out=u_all, in_=u_all, compare_op=mybir.AluOpType.not_equal,
                fill=0.0, base=1, pattern=[[0, H], [-1, SN]], channel_multiplier=1,
            )

            # ---- 10. softmax normalization (all heads batched) ----
            pm = acts.tile([128, H, SN], F32)
            nc.vector.tensor_tensor(pm, p_all, m8, mybir.AluOpType.mult)
            z = tmp.tile([128, H], F32, tag="z")
            nc.vector.reduce_sum(z, pm, axis=mybir.AxisListType.X)
            rz = tmp.tile([128, H], F32, tag="rz")
            nc.vector.reciprocal(rz, z)
            a_all = acts.tile([128, H, SN], F32)
            nc.vector.tensor_tensor(
                a_all, pm, rz[:, :, None].to_broadcast([128, H, SN]),
                mybir.AluOpType.mult,
            )
            # row-128
            pm1 = tmp.tile([1, H, SN], F32, tag="pm1")
            nc.vector.tensor_tensor(pm1, p1_all, m1, mybir.AluOpType.mult)
            z1 = tmp.tile([1, H], F32, tag="z1")
            nc.vector.reduce_sum(z1, pm1, axis=mybir.AxisListType.X)
            rz1 = tmp.tile([1, H], F32, tag="rz1")
            nc.vector.reciprocal(rz1, z1)
            a1 = tmp.tile([1, H, SN], F32, tag="a1")
            nc.vector.tensor_tensor(
                a1, pm1, rz1[:, :, None].to_broadcast([1, H, SN]),
                mybir.AluOpType.mult,
            )

            # ---- 11. attn output DMAs ----
            for h in range(H):
                nc.sync.dma_start(attn_d[h, 0:S, :], a_all[:, h, :])
            nc.sync.dma_start(
                attn_d[:, S : S + 1, :].rearrange("h u j -> u h j"), a1
            )

            # ---- 12. ctx = attn @ v  (via unnormalized u, then scale) ----
            ps_c0 = psA.tile([128, D], F32, tag="psA")
            ps_c1 = psC.tile([1, 256], F32, tag="psC")
            for h in range(H):
                hs = slice(h * HD, (h + 1) * HD)
                nc.tensor.matmul(
                    ps_c0[:, hs], u_all[:, h, 0:S], v_bf[:, hs],
                    start=True, stop=True,
                )
                nc.tensor.matmul(
                    ps_c1[0:1, hs], u_all[:, h, S:SN], v_bf[:, hs],
                    start=True, stop=True,
                )
            ctx0 = acts.tile([128, D], BF)
            nc.vector.tensor_tensor(
                ctx0.rearrange("p (h e) -> p h e", h=H),
                ps_c0.rearrange("p (h e) -> p h e", h=H),
                rz[:, :, None].to_broadcast([128, H, HD]),
                mybir.AluOpType.mult,
            )
            ctx1 = acts.tile([1, D], BF)
            nc.vector.tensor_tensor(
                ctx1.rearrange("p (h e) -> p h e", h=H),
                ps_c1.rearrange("p (h e) -> p h e", h=H),
                rz1[:, :, None].to_broadcast([1, H, HD]),
                mybir.AluOpType.mult,
            )

            # ---- 13. ctxT via PE transposes ----
            ctxT = acts.tile([128, 2, SN], BF)
            for c in range(2):
                cs = slice(c * 128, (c + 1) * 128)
                ps_t0 = psB.tile([128, 256], BF, tag="psB")
                nc.tensor.transpose(ps_t0[:, 0:128], ctx0[:, cs], ident_bf)
                nc.vector.tensor_copy(ctxT[:, c, 0:S], ps_t0[:, 0:128])
                ps_t1 = psB.tile([128, 256], BF, tag="psB")
                nc.tensor.transpose(
                    ps_t1[0:128, 0:1], ctx1[0:1, cs], ident_bf[0:1, 0:1]
                )
                nc.vector.tensor_copy(ctxT[:, c, S:SN], ps_t1[0:128, 0:1])

            # ---- 14. outT = (ctx @ Wo + bo)^T  (feature-major [d', i]) ----
            for c in range(2):
                ps_o = psB.tile([128, 129], F32, tag="psB")
                for k in range(2):
                    nc.tensor.matmul(
                        ps_o[:, 0:SN],
                        wsb[:, O_WO + k, c * 128 : (c + 1) * 128],
                        ctxT[:, k, :],
                        start=(k == 0), stop=False,
                    )
                nc.tensor.matmul(
                    ps_o[:, 0:SN],
                    brows[0:1, R_BO, c * 128 : (c + 1) * 128],
                    ones_bf[0:1, 0:SN],
                    start=False, stop=True,
                )
                o_sb = tmp.tile([128, SN], F32, tag="osb")
                nc.vector.tensor_copy(o_sb, ps_o[:, 0:SN])
                nc.sync.dma_start(outT_d[c * 128 : (c + 1) * 128, :], o_sb)

    if not nc.is_finalized():
        nc.finalize()
    _NC_CACHE = nc
    return nc


def kernel(desc_embeddings, name_embeddings, value_embeddings, cls_token, params):
    name = np.asarray(name_embeddings, np.float32)
    value = np.asarray(value_embeddings, np.float32)
    cls = np.asarray(cls_token, np.float32).reshape(1, D)
    p = {k: np.asarray(v, np.float32) for k, v in params.items()}

    isq = 1.0 / np.sqrt(np.float32(HD))
    wall = np.concatenate(
        [
            p["sfW1"],           # [512,256]
            p["sfW2"],           # [256,256]
            p["Wq"] * isq,
            p["Wk"],
            p["Wv"],
            p["Wo"],
        ],
        axis=0,
    ).astype(BF16)  # [1792, 256]
    brows = np.stack(
        [p["sfb1"], p["sfb2"], p["bq"] * isq, p["bk"], p["bv"], p["bo"]]
    ).astype(BF16)  # [6,256]
    gb = np.stack([p["sfg"], p["sfbeta"]]).astype(np.float32)  # [2,256]
    clsT = cls.T.astype(BF16)  # [256,1]

    nc = _build_nc()
    in_maps = []
    for b in range(B):
        nvT = np.concatenate([name[b].T, value[b].T], axis=0).astype(BF16)  # [512,128]
        in_maps.append(
            {"nvT": nvT, "clsT": clsT, "Wall": wall, "brows": brows, "gb": gb}
        )
    res = run_bass_kernel_spmd(nc, in_maps, core_ids=list(range(NCORES)))

    out = np.empty((B, SN, D), np.float32)
    attn = np.empty((B, H, SN, SN), np.float32)
    for b in range(B):
        out[b] = np.ascontiguousarray(res.results[b]["outT"].T)
        attn[b] = res.results[b]["attn"]
    return out, attn


# revision 20
# speedup vs baseline: 1.1485x; 1.1485x over previous
"""Trainium2 Bass kernel for nn_AdaptiveGraphAttention (B=8,S=128,D=256,H=8).

Data-parallel: 1 sample per NeuronCore (8 cores). Per-core program:
  nv-MLP (+LayerNorm+ReLU) -> nvf ; tok=[cls;nvf] ; Q/K/V projections ;
  masked 8-head attention ; output projection.
Dead code skipped: the edge-MLP result is deleted in the reference, and
gt/aw/adj only feed a `new_adj == 0` mask which is structural (diag, col 0,
and [0,0]) because sigmoid products cannot underflow to exactly 0 for
randn-scale inputs (|logit| << 103).

Layouts: activations feature-major [D,S] where possible (host pre-transposes
inputs); scores computed in both orientations via swapped matmuls so the
attn.T @ v contraction needs no on-device transposes. Softmax uses
unnormalized exp (scores are O(1), no overflow) with multiplicative masks.
Attention row i=128 (the +1 beyond the 128-partition tile) is recovered from
columns of the reverse-orientation exp(scores) via tiny PE transposes.
"""

import sys

sys.path.insert(0, "/opt/trn_rl_repo")

import numpy as np
import ml_dtypes

import concourse.bass as bass
from concourse import bacc
import concourse.mybir as mybir
from concourse.bass_utils import run_bass_kernel_spmd
from concourse.tile import TileContext, ScopedClock
from concourse.masks import make_identity

BF16 = ml_dtypes.bfloat16
F32 = mybir.dt.float32
BF = mybir.dt.bfloat16
I32 = mybir.dt.int32
Alu = mybir.AluOpType
Act = mybir.ActivationFunctionType

B, S, D = 8, 128, 256
H, HD = 8, 32
SN = S + 1  # 129
NCORES = 8

# W_all row-chunk indices (each chunk = 128 rows of the stacked weight matrix)
O_SFW1 = 0  # 4 chunks   [512,256]
O_SFW2 = 4  # 2 chunks
O_WQ = 6  # 2 chunks (pre-scaled by 1/sqrt(HD))
O_WK = 8
O_WV = 10
O_WO = 12
N_WCHUNK = 14
# brows rows
R_SFB1, R_SFB2, R_BQ, R_BK, R_BV, R_BO = range(6)

_NC_CACHE = None


def _build_nc():
    global _NC_CACHE
    if _NC_CACHE is not None:
        return _NC_CACHE
    nc = bacc.Bacc()

    nvT_d = nc.declare_dram_parameter("nvT", [2 * D, S], BF, isOutput=False)
    clsT_d = nc.declare_dram_parameter("clsT", [D, 1], BF, isOutput=False)
    w1_d = nc.declare_dram_parameter("W1", [6 * 128, D], BF, isOutput=False)
    w2_d = nc.declare_dram_parameter("W2", [8 * 128, D], BF, isOutput=False)
    brows_d = nc.declare_dram_parameter("brows", [6, D], BF, isOutput=False)
    gb_d = nc.declare_dram_parameter("gb", [2, D], F32, isOutput=False)
    attn_d = nc.declare_dram_parameter("attn", [H, SN, SN], F32, isOutput=True)
    outT_d = nc.declare_dram_parameter("outT", [D, SN], F32, isOutput=True)

    with TileContext(nc) as tc:
        with (
            tc.tile_pool(name="const", bufs=1) as constp,
            tc.tile_pool(name="acts", bufs=1) as acts,
            tc.tile_pool(name="tmp", bufs=2) as tmp,
            tc.tile_pool(name="psA", bufs=2, space="PSUM") as psA,
            tc.tile_pool(name="psB", bufs=4, space="PSUM") as psB,
            tc.tile_pool(name="psC", bufs=1, space="PSUM") as psC,
        ):
            # ---- input DMAs first (spread across engine queues) ----
            wsb = constp.tile([128, N_WCHUNK, D], BF)
            nc.sync.dma_start(
                wsb[:, 0:6, :], w1_d.rearrange("(o p) n -> p o n", p=128)
            )
            nvT = constp.tile([128, 4, S], BF)
            nc.gpsimd.dma_start(nvT, nvT_d.rearrange("(o p) s -> p o s", p=128))
            brows = constp.tile([1, 6, D], BF)
            nc.scalar.dma_start(brows, brows_d.rearrange("(u r) n -> u r n", u=1))
            gbb = constp.tile([128, 2, D], F32)
            gb_ap = gb_d[:, :]
            gb_bcast = bass.AP(
                tensor=gb_ap.tensor,
                offset=gb_ap.offset,
                ap=[[0, 128], list(gb_ap.ap[0]), list(gb_ap.ap[1])],
            )
            nc.gpsimd.dma_start(gbb, gb_bcast)
            tokT = acts.tile([128, 2, SN], BF)
            nc.gpsimd.dma_start(
                tokT[:, :, 0:1], clsT_d.rearrange("(o p) u -> p o u", p=128)
            )
            nc.sync.dma_start(
                wsb[:, 6:14, :], w2_d.rearrange("(o p) n -> p o n", p=128)
            )
            g_sb = gbb[:, 0, :]
            beta_sb = gbb[:, 1, :]

            # ---- constants (no DMA needed) ----
            ones_bf = constp.tile([1, D], BF)
            nc.vector.memset(ones_bf, 1.0)
            magic = constp.tile([128, 1], I32)
            nc.vector.memset(magic, 0x5F3759DF)
            ident_bf = constp.tile([128, 128], BF)
            make_identity(nc, ident_bf)
            # forward mask (multiplicative): [i, h, j] zero at j==0 and j==i
            m8 = constp.tile([128, H, SN], F32)
            nc.gpsimd.memset(m8, 1.0)
            nc.gpsimd.affine_select(
                out=m8, in_=m8, compare_op=Alu.not_equal,
                fill=0.0, base=0, pattern=[[0, H], [-1, SN]], channel_multiplier=1,
            )  # zero where i - j == 0
            nc.gpsimd.affine_select(
                out=m8, in_=m8, compare_op=Alu.not_equal,
                fill=0.0, base=0, pattern=[[0, H], [1, SN]], channel_multiplier=0,
            )  # zero where j == 0
            # reverse mask (bf16): u[j', h, i] zero at i == j' + 1
            mu = constp.tile([128, H, SN], BF)
            nc.gpsimd.memset(mu, 1.0)
            nc.gpsimd.affine_select(
                out=mu, in_=mu, compare_op=Alu.not_equal,
                fill=0.0, base=1, pattern=[[0, H], [-1, SN]], channel_multiplier=1,
            )

            # ---- 1. h_pre = nv @ sfW1 + sfb1  (token-major [i, d']) ----
            ps_h = psA.tile([128, D], F32, tag="psA")
            for k in range(4):
                nc.tensor.matmul(
                    ps_h, nvT[:, k, :], wsb[:, O_SFW1 + k, :],
                    start=(k == 0), stop=False,
                )
            nc.tensor.matmul(
                ps_h, ones_bf[0:1, 0:S], brows[0:1, R_SFB1, :],
                start=False, stop=True,
            )

            # ---- 2. LayerNorm + ReLU -> h_bf ----
            h_sb = tmp.tile([128, D], F32, tag="hsb")
            r1 = tmp.tile([128, 1], F32, tag="s2")
            nc.scalar.activation(h_sb, ps_h, Act.Copy, accum_out=r1)
            sq_scratch = tmp.tile([128, D], F32, tag="sq")
            nc.vector.tensor_tensor(sq_scratch, ps_h, h_sb, Alu.mult)
            ex2r = tmp.tile([128, 1], F32, tag="s1")
            nc.vector.reduce_sum(ex2r, sq_scratch, axis=mybir.AxisListType.X)
            mean = tmp.tile([128, 1], F32, tag="s3")
            nc.vector.tensor_scalar_mul(mean, r1, 1.0 / D)
            m2e = tmp.tile([128, 1], F32, tag="s4")
            nc.vector.tensor_tensor(m2e, mean, mean, Alu.mult)
            nc.vector.tensor_scalar(m2e, m2e, 1e-5, None, Alu.subtract)
            vpe = tmp.tile([128, 1], F32, tag="s5")
            nc.vector.tensor_scalar(
                vpe, ex2r, 1.0 / D, m2e, Alu.mult, Alu.subtract
            )  # var + eps
            # Newton rsqrt (no ACT Sqrt -> keeps ScalarE on the Exp table only)
            yi = tmp.tile([128, 1], I32, tag="s7")
            nc.vector.tensor_scalar(
                yi, vpe.bitcast(I32), 1, None, Alu.logical_shift_right
            )
            nc.vector.tensor_tensor(yi, magic, yi, Alu.subtract)
            y = yi.bitcast(F32)
            t1 = tmp.tile([128, 1], F32, tag="s8")
            for _ in range(2):
                nc.vector.tensor_tensor(t1, y, y, Alu.mult)
                nc.vector.tensor_tensor(t1, t1, vpe, Alu.mult)
                nc.vector.tensor_scalar(t1, t1, -0.5, 1.5, Alu.mult, Alu.add)
                nc.vector.tensor_tensor(y, y, t1, Alu.mult)
            xn = tmp.tile([128, D], F32, tag="xn")
            nc.vector.tensor_scalar(xn, h_sb, mean, y, Alu.subtract, Alu.mult)
            nc.vector.tensor_tensor(xn, xn, g_sb, Alu.mult)
            nc.vector.tensor_tensor(xn, xn, beta_sb, Alu.add)
            h_bf = acts.tile([128, D], BF)
            nc.vector.tensor_scalar_max(h_bf, xn, 0.0)

            # ---- 3. hT via PE transpose ----
            hT = acts.tile([128, 2, S], BF)
            for c in range(2):
                ps_t = psB.tile([128, 258], BF, tag="psB")
                nc.tensor.transpose(
                    ps_t[:, 0:128], h_bf[:, c * 128 : (c + 1) * 128], ident_bf
                )
                nc.scalar.activation(hT[:, c, :], ps_t[:, 0:128], Act.Copy)

            # ---- 4. nvfT -> tokT[:, :, 1:]  (feature-major) ----
            for c in range(2):
                ps_n = psB.tile([128, 258], F32, tag="psB")
                for k in range(2):
                    nc.tensor.matmul(
                        ps_n[:, 0:S],
                        wsb[:, O_SFW2 + k, c * 128 : (c + 1) * 128],
                        hT[:, k, :],
                        start=(k == 0), stop=False,
                    )
                nc.tensor.matmul(
                    ps_n[:, 0:S],
                    brows[0:1, R_SFB2, c * 128 : (c + 1) * 128],
                    ones_bf[0:1, 0:S],
                    start=False, stop=True,
                )
                eng = nc.vector if c == 0 else nc.scalar
                if c == 0:
                    nc.vector.tensor_copy(tokT[:, c, 1:SN], ps_n[:, 0:S])
                else:
                    nc.scalar.activation(tokT[:, c, 1:SN], ps_n[:, 0:S], Act.Copy)

            # ---- 5. qT / kT (feature-major, head-split: [32, h, i]) ----
            qT = acts.tile([HD, H, SN], BF)
            kT = acts.tile([HD, H, SN], BF)
            for di, (dst, oW, rB) in enumerate(
                ((qT, O_WQ, R_BQ), (kT, O_WK, R_BK))
            ):
                for c in range(2):
                    ps_p = psB.tile([128, 258], F32, tag="psB")
                    for k in range(2):
                        nc.tensor.matmul(
                            ps_p[:, 0:SN],
                            wsb[:, oW + k, c * 128 : (c + 1) * 128],
                            tokT[:, k, :],
                            start=(k == 0), stop=False,
                        )
                    nc.tensor.matmul(
                        ps_p[:, 0:SN],
                        brows[0:1, rB, c * 128 : (c + 1) * 128],
                        ones_bf[0:1, 0:SN],
                        start=False, stop=True,
                    )
                    for hl in range(4):
                        src = ps_p[hl * HD : (hl + 1) * HD, 0:SN]
                        if di == 0:
                            nc.vector.tensor_copy(dst[:, c * 4 + hl, :], src)
                        else:
                            nc.scalar.activation(dst[:, c * 4 + hl, :], src, Act.Copy)

            # ---- 6. v (token-major [j, d'], tokens 1..128 only) ----
            ps_v = psA.tile([128, D], F32, tag="psA")
            for k in range(2):
                nc.tensor.matmul(
                    ps_v, tokT[:, k, 1:SN], wsb[:, O_WV + k, :],
                    start=(k == 0), stop=False,
                )
            nc.tensor.matmul(
                ps_v, ones_bf[0:1, 0:S], brows[0:1, R_BV, :],
                start=False, stop=True,
            )
            v_bf = acts.tile([128, D], BF)
            nc.vector.tensor_copy(v_bf, ps_v)

            # ---- 7. scores, head-pair batched: forward exp + mask-mult,
            #         reverse exp into u ----
            p_all = acts.tile([128, H, SN], F32)
            pm = acts.tile([128, H, SN], F32)
            u_all = acts.tile([128, H, SN], BF)
            for h0 in range(0, H, 2):
                # forward pair: scores[i, j] for heads h0, h0+1
                ps_f = psB.tile([128, 258], F32, tag="psB")
                for g in range(2):
                    qs = qT[:, h0 + g, :]
                    ks = kT[:, h0 + g, :]
                    nc.tensor.matmul(
                        ps_f[:, g * SN : (g + 1) * SN], qs[:, 0:S], ks,
                        start=True, stop=True,
                    )
                nc.scalar.activation(
                    p_all[:, h0 : h0 + 2, :].rearrange("p g j -> p (g j)"),
                    ps_f[:, 0 : 2 * SN],
                    Act.Exp,
                )
                nc.vector.tensor_tensor(
                    pm[:, h0 : h0 + 2, :], p_all[:, h0 : h0 + 2, :],
                    m8[:, h0 : h0 + 2, :], Alu.mult,
                )
                # reverse pair: u[j', i] = exp(scores[i, j'+1])
                ps_r = psB.tile([128, 258], F32, tag="psB")
                for g in range(2):
                    qs = qT[:, h0 + g, :]
                    ks = kT[:, h0 + g, :]
                    nc.tensor.matmul(
                        ps_r[:, g * SN : (g + 1) * SN], ks[:, 1:SN], qs,
                        start=True, stop=True,
                    )
                nc.scalar.activation(
                    u_all[:, h0 : h0 + 2, :].rearrange("p g j -> p (g j)"),
                    ps_r[:, 0 : 2 * SN],
                    Act.Exp,
                )
            # mask u (self-attention) with bf16 multiply on DVE
            nc.vector.tensor_tensor(u_all, u_all, mu, Alu.mult)

            # ---- 8. softmax normalization (rows 0..127, all heads) ----
            z = tmp.tile([128, H], F32, tag="z")
            nc.vector.reduce_sum(z, pm, axis=mybir.AxisListType.X)
            rz = tmp.tile([128, H], F32, tag="rz")
            nc.vector.reciprocal(rz, z)
            a_all = acts.tile([128, H, SN], F32)
            nc.vector.tensor_tensor(
                a_all, pm, rz[:, :, None].to_broadcast([128, H, SN]), Alu.mult
            )
            nc.gpsimd.dma_start(
                attn_d[:, 0:S, :].rearrange("h i j -> i h j"), a_all
            )

            # ---- 9. attention row i=128 from columns of u ----
            t1row = tmp.tile([1, H, S], F32, tag="t1row")
            for half in range(2):
                ps_1 = psC.tile([1, 512], F32, tag="ps1t")
                for hl in range(4):
                    h = half * 4 + hl
                    nc.tensor.matmul(
                        ps_1[0:1, hl * S : (hl + 1) * S],
                        u_all[:, h, S : S + 1],
                        ident_bf,
                        start=True, stop=True,
                    )
                nc.vector.tensor_copy(
                    t1row[0:1, half * 4 : half * 4 + 4, :],
                    ps_1.rearrange("u (g s) -> u g s", g=4),
                )
            z1 = tmp.tile([1, H], F32, tag="z1")
            nc.vector.reduce_sum(z1, t1row, axis=mybir.AxisListType.X)
            rz1 = tmp.tile([1, H], F32, tag="rz1")
            nc.vector.reciprocal(rz1, z1)
            a1 = tmp.tile([1, H, SN], F32, tag="a1")
            nc.vector.memset(a1[:, :, 0:1], 0.0)
            nc.vector.tensor_tensor(
                a1[:, :, 1:SN], t1row, rz1[:, :, None].to_broadcast([1, H, S]),
                Alu.mult,
            )
            nc.gpsimd.dma_start(
                attn_d[:, S : S + 1, :].rearrange("h u j -> u h j"), a1
            )

            # ---- 10. ctx = attn @ v  (via unnormalized u, then scale) ----
            ps_c0 = psA.tile([128, D], F32, tag="psA")
            ps_c1 = psC.tile([1, 512], F32, tag="psc1")
            for h in range(H):
                hs = slice(h * HD, (h + 1) * HD)
                nc.tensor.matmul(
                    ps_c0[:, hs], u_all[:, h, 0:S], v_bf[:, hs],
                    start=True, stop=True,
                )
                nc.tensor.matmul(
                    ps_c1[0:1, hs], u_all[:, h, S:SN], v_bf[:, hs],
                    start=True, stop=True,
                )
            ctx0 = acts.tile([128, D], BF)
            nc.vector.tensor_tensor(
                ctx0.rearrange("p (h e) -> p h e", h=H),
                ps_c0.rearrange("p (h e) -> p h e", h=H),
                rz[:, :, None].to_broadcast([128, H, HD]),
                Alu.mult,
            )
            ctx1 = acts.tile([1, D], BF)
            nc.vector.tensor_tensor(
                ctx1.rearrange("p (h e) -> p h e", h=H),
                ps_c1[0:1, 0:D].rearrange("p (h e) -> p h e", h=H),
                rz1[:, :, None].to_broadcast([1, H, HD]),
                Alu.mult,
            )

            # ---- 11. ctxT via PE transposes ----
            ctxT = acts.tile([128, 2, SN], BF)
            for c in range(2):
                cs = slice(c * 128, (c + 1) * 128)
                ps_t0 = psB.tile([128, 258], BF, tag="psB")
                nc.tensor.transpose(ps_t0[:, 0:128], ctx0[:, cs], ident_bf)
                if c == 0:
                    nc.vector.tensor_copy(ctxT[:, c, 0:S], ps_t0[:, 0:128])
                else:
                    nc.scalar.activation(ctxT[:, c, 0:S], ps_t0[:, 0:128], Act.Copy)
                ps_t1 = psB.tile([128, 258], F32, tag="psB")
                nc.tensor.matmul(
                    ps_t1[0:128, 0:1], ctx1[0:1, cs], ones_bf[0:1, 0:1],
                    start=True, stop=True,
                )
                if c == 0:
                    nc.vector.tensor_copy(ctxT[:, c, S:SN], ps_t1[0:128, 0:1])
                else:
                    nc.scalar.activation(
                        ctxT[:, c, S:SN], ps_t1[0:128, 0:1], Act.Copy
                    )

            # ---- 12. outT = (ctx @ Wo + bo)^T  (feature-major [d', i]) ----
            o_sb = tmp.tile([128, 2, SN], F32, tag="osb")
            for c in range(2):
                ps_o = psB.tile([128, 258], F32, tag="psB")
                for k in range(2):
                    nc.tensor.matmul(
                        ps_o[:, 0:SN],
                        wsb[:, O_WO + k, c * 128 : (c + 1) * 128],
                        ctxT[:, k, :],
                        start=(k == 0), stop=False,
                    )
                nc.tensor.matmul(
                    ps_o[:, 0:SN],
                    brows[0:1, R_BO, c * 128 : (c + 1) * 128],
                    ones_bf[0:1, 0:SN],
                    start=False, stop=True,
                )
                if c == 0:
                    nc.vector.tensor_copy(o_sb[:, c, :], ps_o[:, 0:SN])
                else:
                    nc.scalar.activation(o_sb[:, c, :], ps_o[:, 0:SN], Act.Copy)
            nc.sync.dma_start(
                outT_d.rearrange("(c p) i -> p c i", p=128), o_sb
            )

    if not nc.is_finalized():
        nc.finalize()
    _NC_CACHE = nc
    return nc


def kernel(desc_embeddings, name_embeddings, value_embeddings, cls_token, params):
    name = np.asarray(name_embeddings, np.float32)
    value = np.asarray(value_embeddings, np.float32)
    cls = np.asarray(cls_token, np.float32).reshape(1, D)
    p = {k: np.asarray(v, np.float32) for k, v in params.items()}

    isq = 1.0 / np.sqrt(np.float32(HD))
    w1 = np.concatenate([p["sfW1"], p["sfW2"]], axis=0).astype(BF16)  # [768,256]
    w2 = np.concatenate(
        [p["Wq"] * isq, p["Wk"], p["Wv"], p["Wo"]], axis=0
    ).astype(BF16)  # [1024,256]
    brows = np.stack(
        [p["sfb1"], p["sfb2"], p["bq"] * isq, p["bk"], p["bv"], p["bo"]]
    ).astype(BF16)  # [6,256]
    gb = np.stack([p["sfg"], p["sfbeta"]]).astype(np.float32)  # [2,256]
    clsT = cls.T.astype(BF16)  # [256,1]

    nc = _build_nc()
    in_maps = []
    for b in range(B):
        nvT = np.concatenate([name[b].T, value[b].T], axis=0).astype(BF16)  # [512,128]
        in_maps.append(
            {"nvT": nvT, "clsT": clsT, "W1": w1, "W2": w2, "brows": brows, "gb": gb}
        )
    res = run_bass_kernel_spmd(nc, in_maps, core_ids=list(range(NCORES)))

    out = np.empty((B, SN, D), np.float32)
    attn = np.empty((B, H, SN, SN), np.float32)
    for b in range(B):
        out[b] = np.ascontiguousarray(res.results[b]["outT"].T)
        attn[b] = res.results[b]["attn"]
    return out, attn


# revision 22
# speedup vs baseline: 1.4002x; 1.2192x over previous
"""Trainium2 Bass kernel for nn_AdaptiveGraphAttention (B=8,S=128,D=256,H=8).

Data-parallel: 1 sample per NeuronCore (8 cores). Per-core program:
  nv-MLP (+LayerNorm+ReLU) -> nvf ; tok=[cls;nvf] ; Q/K/V projections ;
  masked 8-head attention ; output projection.
Dead code skipped: the edge-MLP result is deleted in the reference, and
gt/aw/adj only feed a `new_adj == 0` mask which is structural (diag, col 0,
and [0,0]) because sigmoid products cannot underflow to exactly 0 for
randn-scale inputs (|logit| << 103).

Layouts: activations feature-major [D,S] where possible (host pre-transposes
inputs); scores computed in both orientations via swapped matmuls so the
attn.T @ v contraction needs no on-device transposes. Softmax uses
unnormalized exp (scores are O(1), no overflow) with multiplicative masks.
Attention row i=128 (the +1 beyond the 128-partition tile) is recovered from
columns of the reverse-orientation exp(scores) via tiny PE transposes.
"""

import sys

sys.path.insert(0, "/opt/trn_rl_repo")

import numpy as np
import ml_dtypes

import concourse.bass as bass
from concourse import bacc
import concourse.mybir as mybir
from concourse.bass_utils import run_bass_kernel_spmd
from concourse.tile import TileContext, ScopedClock
from concourse.masks import make_identity

BF16 = ml_dtypes.bfloat16
F32 = mybir.dt.float32
BF = mybir.dt.bfloat16
I32 = mybir.dt.int32
Alu = mybir.AluOpType
Act = mybir.ActivationFunctionType

B, S, D = 8, 128, 256
H, HD = 8, 32
SN = S + 1  # 129
NCORES = 8

# W_all row-chunk indices (each chunk = 128 rows of the stacked weight matrix)
O_SFW1 = 0  # 4 chunks   [512,256]
O_SFW2 = 4  # 2 chunks
O_WQ = 6  # 2 chunks (pre-scaled by 1/sqrt(HD))
O_WK = 8
O_WV = 10
O_WO = 12
N_WCHUNK = 14
# brows rows
R_SFB1, R_SFB2, R_BQ, R_BK, R_BV, R_BO = range(6)

_NC_CACHE = None


def _build_nc():
    global _NC_CACHE
    if _NC_CACHE is not None:
        return _NC_CACHE
    nc = bacc.Bacc()

    nvT_d = nc.declare_dram_parameter("nvT", [2 * D, S], BF, isOutput=False)
    clsT_d = nc.declare_dram_parameter("clsT", [D, 1], BF, isOutput=False)
    w1_d = nc.declare_dram_parameter("W1", [6 * 128, D], BF, isOutput=False)
    w2_d = nc.declare_dram_parameter("W2", [8 * 128, D], BF, isOutput=False)
    brows_d = nc.declare_dram_parameter("brows", [6, D], BF, isOutput=False)
    gb_d = nc.declare_dram_parameter("gb", [2, D], F32, isOutput=False)
    attn_d = nc.declare_dram_parameter("attn", [SN, H, SN], F32, isOutput=True)
    outT_d = nc.declare_dram_parameter("outT", [D, SN], F32, isOutput=True)

    with TileContext(nc) as tc:
        with (
            tc.tile_pool(name="const", bufs=1) as constp,
            tc.tile_pool(name="acts", bufs=1) as acts,
            tc.tile_pool(name="tmp", bufs=2) as tmp,
            tc.tile_pool(name="psA", bufs=2, space="PSUM") as psA,
            tc.tile_pool(name="psB", bufs=4, space="PSUM") as psB,
            tc.tile_pool(name="psC", bufs=1, space="PSUM") as psC,
        ):
            # ---- input DMAs first (spread across engine queues) ----
            wsb = constp.tile([128, N_WCHUNK, D], BF)
            nc.sync.dma_start(
                wsb[:, 0:6, :], w1_d.rearrange("(o p) n -> p o n", p=128)
            )
            nvT = constp.tile([128, 4, S], BF)
            nc.gpsimd.dma_start(nvT, nvT_d.rearrange("(o p) s -> p o s", p=128))
            brows = constp.tile([1, 6, D], BF)
            nc.scalar.dma_start(brows, brows_d.rearrange("(u r) n -> u r n", u=1))
            gbb = constp.tile([128, 2, D], F32)
            gb_ap = gb_d[:, :]
            gb_bcast = bass.AP(
                tensor=gb_ap.tensor,
                offset=gb_ap.offset,
                ap=[[0, 128], list(gb_ap.ap[0]), list(gb_ap.ap[1])],
            )
            nc.gpsimd.dma_start(gbb, gb_bcast)
            tokT = acts.tile([128, 2, SN], BF)
            nc.gpsimd.dma_start(
                tokT[:, :, 0:1], clsT_d.rearrange("(o p) u -> p o u", p=128)
            )
            nc.sync.dma_start(
                wsb[:, 6:14, :], w2_d.rearrange("(o p) n -> p o n", p=128)
            )
            g_sb = gbb[:, 0, :]
            beta_sb = gbb[:, 1, :]

            # ---- constants (no DMA needed) ----
            ones_bf = constp.tile([1, D], BF)
            nc.vector.memset(ones_bf, 1.0)
            magic = constp.tile([128, 1], I32)
            nc.vector.memset(magic, 0x5F3759DF)
            ident_bf = constp.tile([128, 128], BF)
            make_identity(nc, ident_bf)
            # forward mask (multiplicative): [i, h, j] zero at j==0 and j==i
            m8 = constp.tile([128, H, SN], F32)
            nc.gpsimd.memset(m8, 1.0)
            nc.gpsimd.affine_select(
                out=m8, in_=m8, compare_op=Alu.not_equal,
                fill=0.0, base=0, pattern=[[0, H], [-1, SN]], channel_multiplier=1,
            )  # zero where i - j == 0
            nc.gpsimd.affine_select(
                out=m8, in_=m8, compare_op=Alu.not_equal,
                fill=0.0, base=0, pattern=[[0, H], [1, SN]], channel_multiplier=0,
            )  # zero where j == 0
            # reverse mask (bf16): u[j', h, i] zero at i == j' + 1
            mu = constp.tile([128, H, SN], BF)
            nc.gpsimd.memset(mu, 1.0)
            nc.gpsimd.affine_select(
                out=mu, in_=mu, compare_op=Alu.not_equal,
                fill=0.0, base=1, pattern=[[0, H], [-1, SN]], channel_multiplier=1,
            )

            # ---- 1. h_pre = nv @ sfW1 + sfb1  (token-major [i, d']) ----
            ps_h = psA.tile([128, D], F32, tag="psA")
            for k in range(4):
                nc.tensor.matmul(
                    ps_h, nvT[:, k, :], wsb[:, O_SFW1 + k, :],
                    start=(k == 0), stop=False,
                )
            nc.tensor.matmul(
                ps_h, ones_bf[0:1, 0:S], brows[0:1, R_SFB1, :],
                start=False, stop=True,
            )

            # ---- 2. LayerNorm + ReLU -> h_bf ----
            h_sb = tmp.tile([128, D], F32, tag="hsb")
            r1 = tmp.tile([128, 1], F32, tag="s2")
            nc.scalar.activation(h_sb, ps_h, Act.Copy, accum_out=r1)
            sq_scratch = tmp.tile([128, D], F32, tag="sq")
            nc.vector.tensor_tensor(sq_scratch, ps_h, h_sb, Alu.mult)
            ex2r = tmp.tile([128, 1], F32, tag="s1")
            nc.vector.reduce_sum(ex2r, sq_scratch, axis=mybir.AxisListType.X)
            mean = tmp.tile([128, 1], F32, tag="s3")
            nc.vector.tensor_scalar_mul(mean, r1, 1.0 / D)
            m2e = tmp.tile([128, 1], F32, tag="s4")
            nc.vector.tensor_scalar(
                m2e, mean, mean, 1e-5, Alu.mult, Alu.subtract
            )  # mean^2 - eps
            vpe = tmp.tile([128, 1], F32, tag="s5")
            nc.vector.tensor_scalar(
                vpe, ex2r, 1.0 / D, m2e, Alu.mult, Alu.subtract
            )  # var + eps
            # Newton rsqrt (no ACT Sqrt -> keeps ScalarE on the Exp table only)
            yi = tmp.tile([128, 1], I32, tag="s7")
            nc.vector.tensor_scalar(
                yi, vpe.bitcast(I32), 1, None, Alu.logical_shift_right
            )
            nc.vector.tensor_tensor(yi, magic, yi, Alu.subtract)
            y = yi.bitcast(F32)
            t1 = tmp.tile([128, 1], F32, tag="s8")
            for _ in range(1):
                nc.vector.tensor_tensor(t1, y, y, Alu.mult)
                nc.vector.tensor_tensor(t1, t1, vpe, Alu.mult)
                nc.vector.tensor_scalar(t1, t1, -0.5, 1.5, Alu.mult, Alu.add)
                nc.vector.tensor_tensor(y, y, t1, Alu.mult)
            xn = tmp.tile([128, D], F32, tag="xn")
            nc.vector.tensor_scalar(xn, h_sb, mean, y, Alu.subtract, Alu.mult)
            nc.vector.tensor_tensor(xn, xn, g_sb, Alu.mult)
            nc.vector.tensor_tensor(xn, xn, beta_sb, Alu.add)
            h_bf = acts.tile([128, D], BF)
            nc.vector.tensor_scalar_max(h_bf, xn, 0.0)

            # ---- 3. hT via PE transpose ----
            hT = acts.tile([128, 2, S], BF)
            for c in range(2):
                ps_t = psB.tile([128, 258], BF, tag="psB")
                nc.tensor.transpose(
                    ps_t[:, 0:128], h_bf[:, c * 128 : (c + 1) * 128], ident_bf
                )
                nc.scalar.activation(hT[:, c, :], ps_t[:, 0:128], Act.Copy)

            # ---- 4. nvfT -> tokT[:, :, 1:]  (feature-major) ----
            for c in range(2):
                ps_n = psB.tile([128, 258], F32, tag="psB")
                for k in range(2):
                    nc.tensor.matmul(
                        ps_n[:, 0:S],
                        wsb[:, O_SFW2 + k, c * 128 : (c + 1) * 128],
                        hT[:, k, :],
                        start=(k == 0), stop=False,
                    )
                nc.tensor.matmul(
                    ps_n[:, 0:S],
                    brows[0:1, R_SFB2, c * 128 : (c + 1) * 128],
                    ones_bf[0:1, 0:S],
                    start=False, stop=True,
                )
                eng = nc.vector if c == 0 else nc.scalar
                if c == 0:
                    nc.vector.tensor_copy(tokT[:, c, 1:SN], ps_n[:, 0:S])
                else:
                    nc.scalar.activation(tokT[:, c, 1:SN], ps_n[:, 0:S], Act.Copy)

            # ---- 5. qT / kT (feature-major, head-split: [32, h, i]) ----
            qT = acts.tile([HD, H, SN], BF)
            kT = acts.tile([HD, H, SN], BF)
            for di, (dst, oW, rB) in enumerate(
                ((qT, O_WQ, R_BQ), (kT, O_WK, R_BK))
            ):
                for c in range(2):
                    ps_p = psB.tile([128, 258], F32, tag="psB")
                    for k in range(2):
                        nc.tensor.matmul(
                            ps_p[:, 0:SN],
                            wsb[:, oW + k, c * 128 : (c + 1) * 128],
                            tokT[:, k, :],
                            start=(k == 0), stop=False,
                        )
                    nc.tensor.matmul(
                        ps_p[:, 0:SN],
                        brows[0:1, rB, c * 128 : (c + 1) * 128],
                        ones_bf[0:1, 0:SN],
                        start=False, stop=True,
                    )
                    for hl in range(4):
                        src = ps_p[hl * HD : (hl + 1) * HD, 0:SN]
                        if di == 0:
                            nc.vector.tensor_copy(dst[:, c * 4 + hl, :], src)
                        else:
                            nc.scalar.activation(dst[:, c * 4 + hl, :], src, Act.Copy)

            # ---- 6. v (token-major [j, d'], tokens 1..128 only) ----
            ps_v = psA.tile([128, D], F32, tag="psA")
            for k in range(2):
                nc.tensor.matmul(
                    ps_v, tokT[:, k, 1:SN], wsb[:, O_WV + k, :],
                    start=(k == 0), stop=False,
                )
            nc.tensor.matmul(
                ps_v, ones_bf[0:1, 0:S], brows[0:1, R_BV, :],
                start=False, stop=True,
            )
            v_bf = acts.tile([128, D], BF)
            nc.vector.tensor_copy(v_bf, ps_v)

            # ---- 7. scores, head-pair batched: forward exp + mask-mult,
            #         reverse exp into u ----
            p_all = acts.tile([128, H, SN], F32)
            pm = acts.tile([128, H, SN], F32)
            u_all = acts.tile([128, H, SN], BF)
            for h0 in range(0, H, 2):
                # forward pair: scores[i, j] for heads h0, h0+1
                ps_f = psB.tile([128, 258], F32, tag="psB")
                for g in range(2):
                    qs = qT[:, h0 + g, :]
                    ks = kT[:, h0 + g, :]
                    nc.tensor.matmul(
                        ps_f[:, g * SN : (g + 1) * SN], qs[:, 0:S], ks,
                        start=True, stop=True,
                    )
                nc.scalar.activation(
                    p_all[:, h0 : h0 + 2, :].rearrange("p g j -> p (g j)"),
                    ps_f[:, 0 : 2 * SN],
                    Act.Exp,
                )
                nc.vector.tensor_tensor(
                    pm[:, h0 : h0 + 2, :], p_all[:, h0 : h0 + 2, :],
                    m8[:, h0 : h0 + 2, :], Alu.mult,
                )
                # reverse pair: u[j', i] = exp(scores[i, j'+1])
                ps_r = psB.tile([128, 258], F32, tag="psB")
                for g in range(2):
                    qs = qT[:, h0 + g, :]
                    ks = kT[:, h0 + g, :]
                    nc.tensor.matmul(
                        ps_r[:, g * SN : (g + 1) * SN], ks[:, 1:SN], qs,
                        start=True, stop=True,
                    )
                nc.scalar.activation(
                    u_all[:, h0 : h0 + 2, :].rearrange("p g j -> p (g j)"),
                    ps_r[:, 0 : 2 * SN],
                    Act.Exp,
                )
            # mask u (self-attention) with bf16 multiply on DVE
            nc.vector.tensor_tensor(u_all, u_all, mu, Alu.mult)

            # ---- 8. softmax normalization (rows 0..127, all heads) ----
            a_all_scratch = tmp.tile([128, 4, SN], F32, tag="zscr")
            z = tmp.tile([128, H], F32, tag="z")
            nc.vector.reduce_sum(
                z[:, 0:4], pm[:, 0:4, :], axis=mybir.AxisListType.X
            )
            for h in range(4, H):
                nc.scalar.activation(
                    a_all_scratch[:, h - 4, :], pm[:, h, :], Act.Copy,
                    accum_out=z[:, h : h + 1],
                )
            rz = tmp.tile([128, H], F32, tag="rz")
            nc.vector.reciprocal(rz, z)
            a_all = acts.tile([128, H, SN], F32)
            for h in range(H):
                if h % 2 == 0:
                    nc.scalar.activation(
                        a_all[:, h, :], pm[:, h, :], Act.Copy,
                        scale=rz[:, h : h + 1],
                    )
                else:
                    nc.vector.tensor_scalar_mul(
                        a_all[:, h, :], pm[:, h, :], rz[:, h : h + 1]
                    )
            nc.sync.dma_start(attn_d[0:S, :, :], a_all)

            # ---- 9. attention row i=128 from columns of u ----
            t1row = tmp.tile([1, H, S], F32, tag="t1row")
            for half in range(2):
                ps_1 = psC.tile([1, 512], F32, tag="ps1t")
                for hl in range(4):
                    h = half * 4 + hl
                    nc.tensor.matmul(
                        ps_1[0:1, hl * S : (hl + 1) * S],
                        u_all[:, h, S : S + 1],
                        ident_bf,
                        start=True, stop=True,
                    )
                nc.vector.tensor_copy(
                    t1row[0:1, half * 4 : half * 4 + 4, :],
                    ps_1.rearrange("u (g s) -> u g s", g=4),
                )
            z1 = tmp.tile([1, H], F32, tag="z1")
            nc.vector.reduce_sum(z1, t1row, axis=mybir.AxisListType.X)
            rz1 = tmp.tile([1, H], F32, tag="rz1")
            nc.vector.reciprocal(rz1, z1)
            a1 = tmp.tile([1, H, SN], F32, tag="a1")
            nc.vector.memset(a1[:, :, 0:1], 0.0)
            nc.vector.tensor_tensor(
                a1[:, :, 1:SN], t1row, rz1[:, :, None].to_broadcast([1, H, S]),
                Alu.mult,
            )
            nc.gpsimd.dma_start(attn_d[S : S + 1, :, :], a1)

            # ---- 10. ctx = attn @ v  (via unnormalized u, then scale) ----
            ps_c0 = psA.tile([128, D], F32, tag="psA")
            ps_c1 = psC.tile([1, 512], F32, tag="psc1")
            for h in range(H):
                hs = slice(h * HD, (h + 1) * HD)
                nc.tensor.matmul(
                    ps_c0[:, hs], u_all[:, h, 0:S], v_bf[:, hs],
                    start=True, stop=True,
                )
                nc.tensor.matmul(
                    ps_c1[0:1, hs], u_all[:, h, S:SN], v_bf[:, hs],
                    start=True, stop=True,
                )
            ctx0 = acts.tile([128, D], BF)
            nc.vector.tensor_tensor(
                ctx0.rearrange("p (h e) -> p h e", h=H),
                ps_c0.rearrange("p (h e) -> p h e", h=H),
                rz[:, :, None].to_broadcast([128, H, HD]),
                Alu.mult,
            )
            ctx1 = acts.tile([1, D], BF)
            nc.vector.tensor_tensor(
                ctx1.rearrange("p (h e) -> p h e", h=H),
                ps_c1[0:1, 0:D].rearrange("p (h e) -> p h e", h=H),
                rz1[:, :, None].to_broadcast([1, H, HD]),
                Alu.mult,
            )

            # ---- 11. ctxT via PE transposes ----
            ctxT = acts.tile([128, 2, SN], BF)
            for c in range(2):
                cs = slice(c * 128, (c + 1) * 128)
                ps_t0 = psB.tile([128, 258], BF, tag="psB")
                nc.tensor.transpose(ps_t0[:, 0:128], ctx0[:, cs], ident_bf)
                if c == 0:
                    nc.vector.tensor_copy(ctxT[:, c, 0:S], ps_t0[:, 0:128])
                else:
                    nc.scalar.activation(ctxT[:, c, 0:S], ps_t0[:, 0:128], Act.Copy)
                ps_t1 = psB.tile([128, 258], F32, tag="psB")
                nc.tensor.matmul(
                    ps_t1[0:128, 0:1], ctx1[0:1, cs], ones_bf[0:1, 0:1],
                    start=True, stop=True,
                )
                if c == 0:
                    nc.vector.tensor_copy(ctxT[:, c, S:SN], ps_t1[0:128, 0:1])
                else:
                    nc.scalar.activation(
                        ctxT[:, c, S:SN], ps_t1[0:128, 0:1], Act.Copy
                    )

            # ---- 12. outT = (ctx @ Wo + bo)^T  (feature-major [d', i]) ----
            o_sb = tmp.tile([128, 2, SN], F32, tag="osb")
            for c in range(2):
                ps_o = psB.tile([128, 258], F32, tag="psB")
                for k in range(2):
                    nc.tensor.matmul(
                        ps_o[:, 0:SN],
                        wsb[:, O_WO + k, c * 128 : (c + 1) * 128],
                        ctxT[:, k, :],
                        start=(k == 0), stop=False,
                    )
                nc.tensor.matmul(
                    ps_o[:, 0:SN],
                    brows[0:1, R_BO, c * 128 : (c + 1) * 128],
                    ones_bf[0:1, 0:SN],
                    start=False, stop=True,
                )
                if c == 0:
                    nc.vector.tensor_copy(o_sb[:, c, :], ps_o[:, 0:SN])
                else:
                    nc.scalar.activation(o_sb[:, c, :], ps_o[:, 0:SN], Act.Copy)
            nc.sync.dma_start(
                outT_d.rearrange("(c p) i -> p c i", p=128), o_sb
            )

    if not nc.is_finalized():
        nc.finalize()
    _NC_CACHE = nc
    return nc


def kernel(desc_embeddings, name_embeddings, value_embeddings, cls_token, params):
    name = np.asarray(name_embeddings, np.float32)
    value = np.asarray(value_embeddings, np.float32)
    cls = np.asarray(cls_token, np.float32).reshape(1, D)
    p = {k: np.asarray(v, np.float32) for k, v in params.items()}

    isq = 1.0 / np.sqrt(np.float32(HD))
    w1 = np.concatenate([p["sfW1"], p["sfW2"]], axis=0).astype(BF16)  # [768,256]
    w2 = np.concatenate(
        [p["Wq"] * isq, p["Wk"], p["Wv"], p["Wo"]], axis=0
    ).astype(BF16)  # [1024,256]
    brows = np.stack(
        [p["sfb1"], p["sfb2"], p["bq"] * isq, p["bk"], p["bv"], p["bo"]]
    ).astype(BF16)  # [6,256]
    gb = np.stack([p["sfg"], p["sfbeta"]]).astype(np.float32)  # [2,256]
    clsT = cls.T.astype(BF16)  # [256,1]

    nc = _build_nc()
    in_maps = []
    for b in range(B):
        nvT = np.concatenate([name[b].T, value[b].T], axis=0).astype(BF16)  # [512,128]
        in_maps.append(
            {"nvT": nvT, "clsT": clsT, "W1": w1, "W2": w2, "brows": brows, "gb": gb}
        )
    res = run_bass_kernel_spmd(nc, in_maps, core_ids=list(range(NCORES)))

    out = np.empty((B, SN, D), np.float32)
    attn = np.empty((B, H, SN, SN), np.float32)
    for b in range(B):
        out[b] = np.ascontiguousarray(res.results[b]["outT"].T)
        attn[b] = res.results[b]["attn"].transpose(1, 0, 2)
    return out, attn


# revision 29
# speedup vs baseline: 1.6038x; 1.1454x over previous
"""Trainium2 Bass kernel for nn_AdaptiveGraphAttention (B=8,S=128,D=256,H=8).

Data-parallel: 1 sample per NeuronCore (8 cores). Per-core program:
  nv-MLP (+LayerNorm+ReLU) -> nvf ; tok=[cls;nvf] ; Q/K/V projections ;
  masked 8-head attention ; output projection.
Dead code skipped: the edge-MLP result is deleted in the reference, and
gt/aw/adj only feed a `new_adj == 0` mask which is structural (diag, col 0,
and [0,0]) because sigmoid products cannot underflow to exactly 0 for
randn-scale inputs (|logit| << 103).

Layouts: activations feature-major [D,S] where possible (host pre-transposes
inputs); scores computed in both orientations via swapped matmuls so the
attn.T @ v contraction needs no on-device transposes. Softmax uses
unnormalized exp (scores are O(1), no overflow) with multiplicative masks.
Attention row i=128 (the +1 beyond the 128-partition tile) is recovered from
columns of the reverse-orientation exp(scores) via tiny PE transposes.
"""

import sys

sys.path.insert(0, "/opt/trn_rl_repo")

import numpy as np
import ml_dtypes

import concourse.bass as bass
from concourse import bacc
import concourse.mybir as mybir
from concourse.bass_utils import run_bass_kernel_spmd
from concourse.tile import TileContext, ScopedClock
from concourse.masks import make_identity

BF16 = ml_dtypes.bfloat16
F32 = mybir.dt.float32
BF = mybir.dt.bfloat16
I32 = mybir.dt.int32
Alu = mybir.AluOpType
Act = mybir.ActivationFunctionType

B, S, D = 8, 128, 256
H, HD = 8, 32
SN = S + 1  # 129
NCORES = 8

# W_all row-chunk indices (each chunk = 128 rows of the stacked weight matrix)
O_SFW1 = 0  # 4 chunks   [512,256]
O_SFW2 = 4  # 2 chunks
O_WQ = 6  # 2 chunks (pre-scaled by 1/sqrt(HD))
O_WK = 8
O_WV = 10
O_WO = 12
N_WCHUNK = 14
# brows rows
R_SFB1, R_SFB2, R_BQ, R_BK, R_BV, R_BO = range(6)

_NC_CACHE = None


def _build_nc():
    global _NC_CACHE
    if _NC_CACHE is not None:
        return _NC_CACHE
    nc = bacc.Bacc()

    nvT_d = nc.declare_dram_parameter("nvT", [2 * D, S], BF, isOutput=False)
    clsT_d = nc.declare_dram_parameter("clsT", [D, 1], BF, isOutput=False)
    w1_d = nc.declare_dram_parameter("W1", [6 * 128, D], BF, isOutput=False)
    w2_d = nc.declare_dram_parameter("W2", [8 * 128, D], BF, isOutput=False)
    brows_d = nc.declare_dram_parameter("brows", [6, D], BF, isOutput=False)
    gb_d = nc.declare_dram_parameter("gb", [2, D], F32, isOutput=False)
    bcols_d = nc.declare_dram_parameter("bcols", [128, 6, 2], F32, isOutput=False)
    attn_d = nc.declare_dram_parameter("attn", [SN, H, SN], F32, isOutput=True)
    outT_d = nc.declare_dram_parameter("outT", [D, SN], F32, isOutput=True)

    with TileContext(nc) as tc:
        with (
            tc.tile_pool(name="const", bufs=1) as constp,
            tc.tile_pool(name="acts", bufs=1) as acts,
            tc.tile_pool(name="tmp", bufs=2) as tmp,
            tc.tile_pool(name="psA", bufs=2, space="PSUM") as psA,
            tc.tile_pool(name="psB", bufs=4, space="PSUM") as psB,
            tc.tile_pool(name="psC", bufs=1, space="PSUM") as psC,
        ):
            # ---- input DMAs first (spread across engine queues) ----
            wsb1 = constp.tile([128, 6, D], BF)
            nc.sync.dma_start(wsb1, w1_d.rearrange("(o p) n -> p o n", p=128))
            nvT = constp.tile([128, 4, S], BF)
            nc.gpsimd.dma_start(nvT, nvT_d.rearrange("(o p) s -> p o s", p=128))
            brows = constp.tile([1, 6, D], BF)
            nc.scalar.dma_start(brows, brows_d.rearrange("(u r) n -> u r n", u=1))
            gbb = constp.tile([128, 2, D], F32)
            gb_ap = gb_d[:, :]
            gb_bcast = bass.AP(
                tensor=gb_ap.tensor,
                offset=gb_ap.offset,
                ap=[[0, 128], list(gb_ap.ap[0]), list(gb_ap.ap[1])],
            )
            nc.gpsimd.dma_start(gbb, gb_bcast)
            tokT = acts.tile([128, 2, SN], BF)
            nc.gpsimd.dma_start(
                tokT[:, :, 0:1], clsT_d.rearrange("(o p) u -> p o u", p=128)
            )
            wsb2 = constp.tile([128, 8, D], BF)
            nc.sync.dma_start(wsb2, w2_d.rearrange("(o p) n -> p o n", p=128))
            bcols = constp.tile([128, 6, 2], F32)
            nc.sync.dma_start(bcols, bcols_d[:, :, :])
            g_sb = gbb[:, 0, :]
            beta_sb = gbb[:, 1, :]

            # ---- constants (no DMA needed) ----
            ones_bf = constp.tile([1, D], BF)
            nc.vector.memset(ones_bf, 1.0)
            magic = constp.tile([128, 1], I32)
            nc.vector.memset(magic, 0x5F3759DF)
            ident_bf = constp.tile([128, 128], BF)
            make_identity(nc, ident_bf)
            # forward mask (multiplicative): [i, h, j] zero at j==0 and j==i
            m8 = constp.tile([128, H, SN], F32)
            nc.gpsimd.memset(m8, 1.0)
            nc.gpsimd.affine_select(
                out=m8, in_=m8, compare_op=Alu.not_equal,
                fill=0.0, base=0, pattern=[[0, H], [-1, SN]], channel_multiplier=1,
            )  # zero where i - j == 0
            nc.gpsimd.affine_select(
                out=m8, in_=m8, compare_op=Alu.not_equal,
                fill=0.0, base=0, pattern=[[0, H], [1, SN]], channel_multiplier=0,
            )  # zero where j == 0
            # reverse mask (bf16): u[j', h, i] zero at i == j' + 1
            mu = constp.tile([128, H, SN], BF)
            nc.gpsimd.memset(mu, 1.0)
            nc.gpsimd.affine_select(
                out=mu, in_=mu, compare_op=Alu.not_equal,
                fill=0.0, base=1, pattern=[[0, H], [-1, SN]], channel_multiplier=1,
            )

            # ---- 1. h_pre = nv @ sfW1 + sfb1  (token-major [i, d']) ----
            ps_h = psA.tile([128, D], F32, tag="psA")
            for k in range(4):
                nc.tensor.matmul(
                    ps_h, nvT[:, k, :], wsb1[:, k, :],
                    start=(k == 0), stop=False,
                )
            nc.tensor.matmul(
                ps_h, ones_bf[0:1, 0:S], brows[0:1, R_SFB1, :],
                start=False, stop=True,
            )

            # ---- 2. LayerNorm + ReLU -> h_bf ----
            h_sb = tmp.tile([128, D], F32, tag="hsb")
            r1 = tmp.tile([128, 1], F32, tag="s2")
            nc.scalar.activation(h_sb, ps_h, Act.Copy, accum_out=r1)
            sq_scratch = tmp.tile([128, D], F32, tag="sq")
            nc.vector.tensor_tensor(sq_scratch, ps_h, h_sb, Alu.mult)
            ex2r = tmp.tile([128, 1], F32, tag="s1")
            nc.vector.reduce_sum(ex2r, sq_scratch, axis=mybir.AxisListType.X)
            mean = tmp.tile([128, 1], F32, tag="s3")
            nc.vector.tensor_scalar_mul(mean, r1, 1.0 / D)
            m2e = tmp.tile([128, 1], F32, tag="s4")
            nc.vector.tensor_scalar(
                m2e, mean, mean, 1e-5, Alu.mult, Alu.subtract
            )  # mean^2 - eps
            vpe = tmp.tile([128, 1], F32, tag="s5")
            nc.vector.tensor_scalar(
                vpe, ex2r, 1.0 / D, m2e, Alu.mult, Alu.subtract
            )  # var + eps
            # Newton rsqrt (no ACT Sqrt -> keeps ScalarE on the Exp table only)
            yi = tmp.tile([128, 1], I32, tag="s7")
            nc.vector.tensor_scalar(
                yi, vpe.bitcast(I32), 1, None, Alu.logical_shift_right
            )
            nc.vector.tensor_tensor(yi, magic, yi, Alu.subtract)
            y = yi.bitcast(F32)
            t1 = tmp.tile([128, 1], F32, tag="s8")
            for _ in range(1):
                nc.vector.tensor_tensor(t1, y, y, Alu.mult)
                nc.vector.tensor_tensor(t1, t1, vpe, Alu.mult)
                nc.vector.tensor_scalar(t1, t1, -0.5, 1.5, Alu.mult, Alu.add)
                nc.vector.tensor_tensor(y, y, t1, Alu.mult)
            xn = tmp.tile([128, D], F32, tag="xn")
            nc.vector.tensor_scalar(xn, h_sb, mean, y, Alu.subtract, Alu.mult)
            nc.vector.tensor_tensor(xn, xn, g_sb, Alu.mult)
            nc.vector.tensor_tensor(xn, xn, beta_sb, Alu.add)
            h_bf = acts.tile([128, D], BF)
            nc.vector.tensor_scalar_max(h_bf, xn, 0.0)

            # ---- 3. hT via PE transpose ----
            hT = acts.tile([128, 2, S], BF)
            for c in range(2):
                ps_t = psB.tile([128, 258], BF, tag="psB")
                nc.tensor.transpose(
                    ps_t[:, 0:128], h_bf[:, c * 128 : (c + 1) * 128], ident_bf
                )
                nc.scalar.activation(hT[:, c, :], ps_t[:, 0:128], Act.Copy)

            # ---- 4. nvfT -> tokT[:, :, 1:]  (feature-major) ----
            for c in range(2):
                ps_n = psB.tile([128, 258], F32, tag="psB")
                for k in range(2):
                    nc.tensor.matmul(
                        ps_n[:, 0:S],
                        wsb1[:, 4 + k, c * 128 : (c + 1) * 128],
                        hT[:, k, :],
                        start=(k == 0), stop=(k == 1),
                    )
                nc.vector.tensor_scalar_add(
                    tokT[:, c, 1:SN], ps_n[:, 0:S], bcols[:, R_SFB2, c : c + 1]
                )

            # ---- 5. qT / kT (feature-major, head-split: [32, h, i]) ----
            qT = acts.tile([HD, H, SN], BF)
            kT = acts.tile([HD, H, SN], BF)
            for di, (dst, oW, rB) in enumerate(
                ((qT, O_WQ - 6, R_BQ), (kT, O_WK - 6, R_BK))
            ):
                for c in range(2):
                    ps_p = psB.tile([128, 258], F32, tag="psB")
                    for k in range(2):
                        nc.tensor.matmul(
                            ps_p[:, 0:SN],
                            wsb2[:, oW + k, c * 128 : (c + 1) * 128],
                            tokT[:, k, :],
                            start=(k == 0), stop=(k == 1),
                        )
                    for hl in range(4):
                        sl = slice(hl * HD, (hl + 1) * HD)
                        srcap = ps_p[sl, 0:SN]
                        if di == 0:
                            nc.vector.tensor_scalar_add(
                                dst[:, c * 4 + hl, :], srcap, bcols[sl, rB, c : c + 1]
                            )
                        else:
                            nc.scalar.activation(
                                dst[:, c * 4 + hl, :], srcap, Act.Identity,
                                bias=bcols[sl, rB, c : c + 1],
                            )

            # ---- 6. v (token-major [j, d'], tokens 1..128 only) ----
            ps_v = psA.tile([128, D], F32, tag="psA")
            for k in range(2):
                nc.tensor.matmul(
                    ps_v, tokT[:, k, 1:SN], wsb2[:, O_WV - 6 + k, :],
                    start=(k == 0), stop=False,
                )
            nc.tensor.matmul(
                ps_v, ones_bf[0:1, 0:S], brows[0:1, R_BV, :],
                start=False, stop=True,
            )
            v_bf = acts.tile([128, D], BF)
            nc.vector.tensor_copy(v_bf, ps_v)

            # ---- 7. scores, head-pair batched: forward exp + mask-mult,
            #         reverse exp into u ----
            p_all = acts.tile([128, H, SN], F32)
            pm = acts.tile([128, H, SN], F32)
            u_all = acts.tile([128, H, SN], BF)
            def hsl(t, h):
                return t[:, h, :]

            for h0 in range(0, H, 2):
                # forward pair: scores[i, j] for heads h0, h0+1
                ps_f = psB.tile([128, 258], F32, tag="psB")
                for g in range(2):
                    qs = hsl(qT, h0 + g)
                    ks = hsl(kT, h0 + g)
                    nc.tensor.matmul(
                        ps_f[:, g * SN : (g + 1) * SN], qs[:, 0:S], ks,
                        start=True, stop=True,
                    )
                nc.scalar.activation(
                    p_all[:, h0 : h0 + 2, :].rearrange("p g j -> p (g j)"),
                    ps_f[:, 0 : 2 * SN],
                    Act.Exp,
                )
                nc.vector.tensor_tensor(
                    pm[:, h0 : h0 + 2, :], p_all[:, h0 : h0 + 2, :],
                    m8[:, h0 : h0 + 2, :], Alu.mult,
                )
                # reverse pair: u[j', i] = exp(scores[i, j'+1])
                ps_r = psB.tile([128, 258], F32, tag="psB")
                for g in range(2):
                    qs = hsl(qT, h0 + g)
                    ks = hsl(kT, h0 + g)
                    nc.tensor.matmul(
                        ps_r[:, g * SN : (g + 1) * SN], ks[:, 1:SN], qs,
                        start=True, stop=True,
                    )
                nc.scalar.activation(
                    u_all[:, h0 : h0 + 2, :].rearrange("p g j -> p (g j)"),
                    ps_r[:, 0 : 2 * SN],
                    Act.Exp,
                )
            # mask u (self-attention) with bf16 multiply on DVE
            nc.vector.tensor_tensor(u_all, u_all, mu, Alu.mult)

            # ---- 8. softmax normalization (rows 0..127, all heads) ----
            a_all_scratch = tmp.tile([128, 4, SN], F32, tag="zscr")
            z = tmp.tile([128, H], F32, tag="z")
            nc.vector.reduce_sum(
                z[:, 0:4], pm[:, 0:4, :], axis=mybir.AxisListType.X
            )
            for h in range(4, H):
                nc.scalar.activation(
                    a_all_scratch[:, h - 4, :], pm[:, h, :], Act.Copy,
                    accum_out=z[:, h : h + 1],
                )
            rz = tmp.tile([128, H], F32, tag="rz")
            nc.vector.reciprocal(rz, z)
            a_all = acts.tile([128, H, SN], F32)
            for h in range(H):
                if h % 2 == 0:
                    nc.scalar.activation(
                        a_all[:, h, :], pm[:, h, :], Act.Copy,
                        scale=rz[:, h : h + 1],
                    )
                else:
                    nc.vector.tensor_scalar_mul(
                        a_all[:, h, :], pm[:, h, :], rz[:, h : h + 1]
                    )
            nc.sync.dma_start(attn_d[0:S, :, :], a_all)

            # ---- 9. attention row i=128 from columns of u (one matmul) ----
            ps_t8 = psC.tile([8, 512], F32, tag="ps1t")
            ucol = u_all[:, :, S : S + 1].rearrange("p h u -> p (h u)")
            nc.tensor.matmul(ps_t8[:, 0:S], ucol, ident_bf, start=True, stop=True)
            t8 = tmp.tile([8, S], F32, tag="t8")
            nc.vector.tensor_copy(t8, ps_t8[:, 0:S])
            z1p = tmp.tile([8, 1], F32, tag="z1")
            nc.vector.reduce_sum(z1p, t8, axis=mybir.AxisListType.X)
            rz1p = tmp.tile([8, 1], F32, tag="rz1")
            nc.vector.reciprocal(rz1p, z1p)
            a1w = tmp.tile([8, SN], F32, tag="a1")
            nc.vector.memset(a1w[:, 0:1], 0.0)
            nc.vector.tensor_scalar_mul(a1w[:, 1:SN], t8, rz1p)
            nc.scalar.dma_start(attn_d[S, :, :], a1w)
            # rz1 as a free-major row for ctx row-128 normalization
            rz1_bf = tmp.tile([8, 1], BF, tag="rz1b")
            nc.vector.tensor_copy(rz1_bf, rz1p)
            ps_rzr = psC.tile([8, 512], F32, tag="ps1t")
            nc.tensor.matmul(
                ps_rzr[0:1, 0:8], rz1_bf, ident_bf[0:8, 0:8], start=True, stop=True
            )
            rz1 = tmp.tile([1, H], F32, tag="rz1r")
            nc.vector.tensor_copy(rz1, ps_rzr[0:1, 0:8])

            # ---- 10. ctx = attn @ v  (via unnormalized u, then scale) ----
            ps_c0 = psA.tile([128, D], F32, tag="psA")
            ps_c1 = psC.tile([1, 512], F32, tag="psc1")
            for h in range(H):
                hs = slice(h * HD, (h + 1) * HD)
                nc.tensor.matmul(
                    ps_c0[:, hs], u_all[:, h, 0:S], v_bf[:, hs],
                    start=True, stop=True,
                )
                nc.tensor.matmul(
                    ps_c1[0:1, hs], u_all[:, h, S:SN], v_bf[:, hs],
                    start=True, stop=True,
                )
            ctx0 = acts.tile([128, D], BF)
            nc.vector.tensor_tensor(
                ctx0.rearrange("p (h e) -> p h e", h=H),
                ps_c0.rearrange("p (h e) -> p h e", h=H),
                rz[:, :, None].to_broadcast([128, H, HD]),
                Alu.mult,
            )
            ctx1 = acts.tile([1, D], BF)
            nc.vector.tensor_tensor(
                ctx1.rearrange("p (h e) -> p h e", h=H),
                ps_c1[0:1, 0:D].rearrange("p (h e) -> p h e", h=H),
                rz1[:, :, None].to_broadcast([1, H, HD]),
                Alu.mult,
            )

            # ---- 11. ctxT via PE transposes ----
            ctxT = acts.tile([128, 2, SN], BF)
            for c in range(2):
                cs = slice(c * 128, (c + 1) * 128)
                ps_t0 = psB.tile([128, 258], BF, tag="psB")
                nc.tensor.transpose(ps_t0[:, 0:128], ctx0[:, cs], ident_bf)
                if c == 0:
                    nc.vector.tensor_copy(ctxT[:, c, 0:S], ps_t0[:, 0:128])
                else:
                    nc.scalar.activation(ctxT[:, c, 0:S], ps_t0[:, 0:128], Act.Copy)
                ps_t1 = psB.tile([128, 258], F32, tag="psB")
                nc.tensor.matmul(
                    ps_t1[0:128, 0:1], ctx1[0:1, cs], ones_bf[0:1, 0:1],
                    start=True, stop=True,
                )
                if c == 0:
                    nc.vector.tensor_copy(ctxT[:, c, S:SN], ps_t1[0:128, 0:1])
                else:
                    nc.scalar.activation(
                        ctxT[:, c, S:SN], ps_t1[0:128, 0:1], Act.Copy
                    )

            # ---- 12. outT = (ctx @ Wo + bo)^T  (feature-major [d', i]) ----
            o_sb = tmp.tile([128, 2, SN], F32, tag="osb")
            for c in range(2):
                ps_o = psB.tile([128, 258], F32, tag="psB")
                for k in range(2):
                    nc.tensor.matmul(
                        ps_o[:, 0:SN],
                        wsb2[:, O_WO - 6 + k, c * 128 : (c + 1) * 128],
                        ctxT[:, k, :],
                        start=(k == 0), stop=(k == 1),
                    )
                nc.vector.tensor_scalar_add(
                    o_sb[:, c, :], ps_o[:, 0:SN], bcols[:, R_BO, c : c + 1]
                )
            nc.sync.dma_start(
                outT_d.rearrange("(c p) i -> p c i", p=128), o_sb
            )

    if not nc.is_finalized():
        nc.finalize()
    _NC_CACHE = nc
    return nc


def kernel(desc_embeddings, name_embeddings, value_embeddings, cls_token, params):
    name = np.asarray(name_embeddings, np.float32)
    value = np.asarray(value_embeddings, np.float32)
    cls = np.asarray(cls_token, np.float32).reshape(1, D)
    p = {k: np.asarray(v, np.float32) for k, v in params.items()}

    isq = 1.0 / np.sqrt(np.float32(HD))
    w1 = np.concatenate([p["sfW1"], p["sfW2"]], axis=0).astype(BF16)  # [768,256]
    w2 = np.concatenate(
        [p["Wq"] * isq, p["Wk"], p["Wv"], p["Wo"]], axis=0
    ).astype(BF16)  # [1024,256]
    brows = np.stack(
        [p["sfb1"], p["sfb2"], p["bq"] * isq, p["bk"], p["bv"], p["bo"]]
    ).astype(BF16)  # [6,256]
    gb = np.stack([p["sfg"], p["sfbeta"]]).astype(np.float32)  # [2,256]
    brows_f = np.stack(
        [p["sfb1"], p["sfb2"], p["bq"] * isq, p["bk"], p["bv"], p["bo"]]
    ).astype(np.float32)
    bcols = np.ascontiguousarray(
        brows_f.reshape(6, 2, 128).transpose(2, 0, 1)
    )  # [128, 6, 2]
    clsT = cls.T.astype(BF16)  # [256,1]

    nc = _build_nc()
    in_maps = []
    for b in range(B):
        nvT = np.concatenate([name[b].T, value[b].T], axis=0).astype(BF16)  # [512,128]
        in_maps.append(
            {
                "nvT": nvT, "clsT": clsT, "W1": w1, "W2": w2,
                "brows": brows, "gb": gb, "bcols": bcols,
            }
        )
    res = run_bass_kernel_spmd(nc, in_maps, core_ids=list(range(NCORES)))

    out = np.empty((B, SN, D), np.float32)
    attn = np.empty((B, H, SN, SN), np.float32)
    for b in range(B):
        out[b] = np.ascontiguousarray(res.results[b]["outT"].T)
        attn[b] = res.results[b]["attn"].transpose(1, 0, 2)
    return out, attn


# revision 30
# speedup vs baseline: 1.6135x; 1.0061x over previous
"""Trainium2 Bass kernel for nn_AdaptiveGraphAttention (B=8,S=128,D=256,H=8).

Data-parallel: 1 sample per NeuronCore (8 cores). Per-core program:
  nv-MLP (+LayerNorm+ReLU) -> nvf ; tok=[cls;nvf] ; Q/K/V projections ;
  masked 8-head attention ; output projection.
Dead code skipped: the edge-MLP result is deleted in the reference, and
gt/aw/adj only feed a `new_adj == 0` mask which is structural (diag, col 0,
and [0,0]) because sigmoid products cannot underflow to exactly 0 for
randn-scale inputs (|logit| << 103).

Layouts: activations feature-major [D,S] where possible (host pre-transposes
inputs); scores computed in both orientations via swapped matmuls so the
attn.T @ v contraction needs no on-device transposes. Softmax uses
unnormalized exp (scores are O(1), no overflow) with multiplicative masks.
Attention row i=128 (the +1 beyond the 128-partition tile) is recovered from
columns of the reverse-orientation exp(scores) via tiny PE transposes.
"""

import sys

sys.path.insert(0, "/opt/trn_rl_repo")

import numpy as np
import ml_dtypes

import concourse.bass as bass
from concourse import bacc
import concourse.mybir as mybir
from concourse.bass_utils import run_bass_kernel_spmd
from concourse.tile import TileContext, ScopedClock
from concourse.masks import make_identity

BF16 = ml_dtypes.bfloat16
F32 = mybir.dt.float32
BF = mybir.dt.bfloat16
I32 = mybir.dt.int32
Alu = mybir.AluOpType
Act = mybir.ActivationFunctionType

B, S, D = 8, 128, 256
H, HD = 8, 32
SN = S + 1  # 129
NCORES = 8

# W_all row-chunk indices (each chunk = 128 rows of the stacked weight matrix)
O_SFW1 = 0  # 4 chunks   [512,256]
O_SFW2 = 4  # 2 chunks
O_WQ = 6  # 2 chunks (pre-scaled by 1/sqrt(HD))
O_WK = 8
O_WV = 10
O_WO = 12
N_WCHUNK = 14
# brows rows
R_SFB1, R_SFB2, R_BQ, R_BK, R_BV, R_BO = range(6)

_NC_CACHE = None


def _build_nc():
    global _NC_CACHE
    if _NC_CACHE is not None:
        return _NC_CACHE
    nc = bacc.Bacc()

    nvT_d = nc.declare_dram_parameter("nvT", [128, 4, S], BF, isOutput=False)
    clsT_d = nc.declare_dram_parameter("clsT", [128, 2, 1], BF, isOutput=False)
    w1_d = nc.declare_dram_parameter("W1", [128, 6, D], BF, isOutput=False)
    w2_d = nc.declare_dram_parameter("W2", [128, 8, D], BF, isOutput=False)
    brows_d = nc.declare_dram_parameter("brows", [6, D], BF, isOutput=False)
    gb_d = nc.declare_dram_parameter("gb", [2, D], F32, isOutput=False)
    bcols_d = nc.declare_dram_parameter("bcols", [128, 6, 2], F32, isOutput=False)
    attn_d = nc.declare_dram_parameter("attn", [SN, H, SN], F32, isOutput=True)
    outT_d = nc.declare_dram_parameter("outT", [D, SN], F32, isOutput=True)

    with TileContext(nc) as tc:
        with (
            tc.tile_pool(name="const", bufs=1) as constp,
            tc.tile_pool(name="acts", bufs=1) as acts,
            tc.tile_pool(name="tmp", bufs=2) as tmp,
            tc.tile_pool(name="psA", bufs=2, space="PSUM") as psA,
            tc.tile_pool(name="psB", bufs=4, space="PSUM") as psB,
            tc.tile_pool(name="psC", bufs=1, space="PSUM") as psC,
        ):
            # ---- input DMAs first (spread across engine queues) ----
            wsb1 = constp.tile([128, 6, D], BF)
            nc.sync.dma_start(wsb1, w1_d[:, :, :])
            nvT = constp.tile([128, 4, S], BF)
            nc.gpsimd.dma_start(nvT, nvT_d[:, :, :])
            brows = constp.tile([1, 6, D], BF)
            nc.scalar.dma_start(brows, brows_d.rearrange("(u r) n -> u r n", u=1))
            gbb = constp.tile([128, 2, D], F32)
            gb_ap = gb_d[:, :]
            gb_bcast = bass.AP(
                tensor=gb_ap.tensor,
                offset=gb_ap.offset,
                ap=[[0, 128], list(gb_ap.ap[0]), list(gb_ap.ap[1])],
            )
            nc.gpsimd.dma_start(gbb, gb_bcast)
            tokT = acts.tile([128, 2, SN], BF)
            nc.gpsimd.dma_start(tokT[:, :, 0:1], clsT_d[:, :, :])
            wsb2 = constp.tile([128, 8, D], BF)
            nc.sync.dma_start(wsb2, w2_d[:, :, :])
            bcols = constp.tile([128, 6, 2], F32)
            nc.sync.dma_start(bcols, bcols_d[:, :, :])
            g_sb = gbb[:, 0, :]
            beta_sb = gbb[:, 1, :]

            # ---- constants (no DMA needed) ----
            ones_bf = constp.tile([1, D], BF)
            nc.vector.memset(ones_bf, 1.0)
            magic = constp.tile([128, 1], I32)
            nc.vector.memset(magic, 0x5F3759DF)
            ident_bf = constp.tile([128, 128], BF)
            make_identity(nc, ident_bf)
            # forward mask (multiplicative): [i, h, j] zero at j==0 and j==i
            m8 = constp.tile([128, H, SN], F32)
            nc.gpsimd.memset(m8, 1.0)
            nc.gpsimd.affine_select(
                out=m8, in_=m8, compare_op=Alu.not_equal,
                fill=0.0, base=0, pattern=[[0, H], [-1, SN]], channel_multiplier=1,
            )  # zero where i - j == 0
            nc.gpsimd.affine_select(
                out=m8, in_=m8, compare_op=Alu.not_equal,
                fill=0.0, base=0, pattern=[[0, H], [1, SN]], channel_multiplier=0,
            )  # zero where j == 0
            # reverse mask (bf16): u[j', h, i] zero at i == j' + 1
            mu = constp.tile([128, H, SN], BF)
            nc.gpsimd.memset(mu, 1.0)
            nc.gpsimd.affine_select(
                out=mu, in_=mu, compare_op=Alu.not_equal,
                fill=0.0, base=1, pattern=[[0, H], [-1, SN]], channel_multiplier=1,
            )

            # ---- 1. h_pre = nv @ sfW1 + sfb1  (token-major [i, d']) ----
            ps_h = psA.tile([128, D], F32, tag="psA")
            for k in range(4):
                nc.tensor.matmul(
                    ps_h, nvT[:, k, :], wsb1[:, k, :],
                    start=(k == 0), stop=False,
                )
            nc.tensor.matmul(
                ps_h, ones_bf[0:1, 0:S], brows[0:1, R_SFB1, :],
                start=False, stop=True,
            )

            # ---- 2. LayerNorm + ReLU -> h_bf ----
            h_sb = tmp.tile([128, D], F32, tag="hsb")
            r1 = tmp.tile([128, 1], F32, tag="s2")
            nc.scalar.activation(h_sb, ps_h, Act.Copy, accum_out=r1)
            sq_scratch = tmp.tile([128, D], F32, tag="sq")
            nc.vector.tensor_tensor(sq_scratch, ps_h, h_sb, Alu.mult)
            ex2r = tmp.tile([128, 1], F32, tag="s1")
            nc.vector.reduce_sum(ex2r, sq_scratch, axis=mybir.AxisListType.X)
            mean = tmp.tile([128, 1], F32, tag="s3")
            nc.vector.tensor_scalar_mul(mean, r1, 1.0 / D)
            m2e = tmp.tile([128, 1], F32, tag="s4")
            nc.vector.tensor_scalar(
                m2e, mean, mean, 1e-5, Alu.mult, Alu.subtract
            )  # mean^2 - eps
            vpe = tmp.tile([128, 1], F32, tag="s5")
            nc.vector.tensor_scalar(
                vpe, ex2r, 1.0 / D, m2e, Alu.mult, Alu.subtract
            )  # var + eps
            # Newton rsqrt (no ACT Sqrt -> keeps ScalarE on the Exp table only)
            yi = tmp.tile([128, 1], I32, tag="s7")
            nc.vector.tensor_scalar(
                yi, vpe.bitcast(I32), 1, None, Alu.logical_shift_right
            )
            nc.vector.tensor_tensor(yi, magic, yi, Alu.subtract)
            y = yi.bitcast(F32)
            t1 = tmp.tile([128, 1], F32, tag="s8")
            for _ in range(1):
                nc.vector.tensor_tensor(t1, y, y, Alu.mult)
                nc.vector.tensor_tensor(t1, t1, vpe, Alu.mult)
                nc.vector.tensor_scalar(t1, t1, -0.5, 1.5, Alu.mult, Alu.add)
                nc.vector.tensor_tensor(y, y, t1, Alu.mult)
            xn = tmp.tile([128, D], F32, tag="xn")
            nc.vector.tensor_scalar(xn, h_sb, mean, y, Alu.subtract, Alu.mult)
            nc.vector.tensor_tensor(xn, xn, g_sb, Alu.mult)
            nc.vector.tensor_tensor(xn, xn, beta_sb, Alu.add)
            h_bf = acts.tile([128, D], BF)
            nc.vector.tensor_scalar_max(h_bf, xn, 0.0)

            # ---- 3. hT via PE transpose ----
            hT = acts.tile([128, 2, S], BF)
            for c in range(2):
                ps_t = psB.tile([128, 258], BF, tag="psB")
                nc.tensor.transpose(
                    ps_t[:, 0:128], h_bf[:, c * 128 : (c + 1) * 128], ident_bf
                )
                nc.scalar.activation(hT[:, c, :], ps_t[:, 0:128], Act.Copy)

            # ---- 4. nvfT -> tokT[:, :, 1:]  (feature-major) ----
            for c in range(2):
                ps_n = psB.tile([128, 258], F32, tag="psB")
                for k in range(2):
                    nc.tensor.matmul(
                        ps_n[:, 0:S],
                        wsb1[:, 4 + k, c * 128 : (c + 1) * 128],
                        hT[:, k, :],
                        start=(k == 0), stop=(k == 1),
                    )
                nc.vector.tensor_scalar_add(
                    tokT[:, c, 1:SN], ps_n[:, 0:S], bcols[:, R_SFB2, c : c + 1]
                )

            # ---- 5. qT / kT (feature-major, head-split: [32, h, i]) ----
            qT = acts.tile([HD, H, SN], BF)
            kT = acts.tile([HD, H, SN], BF)
            for di, (dst, oW, rB) in enumerate(
                ((qT, O_WQ - 6, R_BQ), (kT, O_WK - 6, R_BK))
            ):
                for c in range(2):
                    ps_p = psB.tile([128, 258], F32, tag="psB")
                    for k in range(2):
                        nc.tensor.matmul(
                            ps_p[:, 0:SN],
                            wsb2[:, oW + k, c * 128 : (c + 1) * 128],
                            tokT[:, k, :],
                            start=(k == 0), stop=(k == 1),
                        )
                    for hl in range(4):
                        sl = slice(hl * HD, (hl + 1) * HD)
                        srcap = ps_p[sl, 0:SN]
                        if di == 0:
                            nc.vector.tensor_scalar_add(
                                dst[:, c * 4 + hl, :], srcap, bcols[sl, rB, c : c + 1]
                            )
                        else:
                            nc.scalar.activation(
                                dst[:, c * 4 + hl, :], srcap, Act.Identity,
                                bias=bcols[sl, rB, c : c + 1],
                            )

            # ---- 6. v (token-major [j, d'], tokens 1..128 only) ----
            ps_v = psA.tile([128, D], F32, tag="psA")
            for k in range(2):
                nc.tensor.matmul(
                    ps_v, tokT[:, k, 1:SN], wsb2[:, O_WV - 6 + k, :],
                    start=(k == 0), stop=False,
                )
            nc.tensor.matmul(
                ps_v, ones_bf[0:1, 0:S], brows[0:1, R_BV, :],
                start=False, stop=True,
            )
            v_bf = acts.tile([128, D], BF)
            nc.vector.tensor_copy(v_bf, ps_v)

            # ---- 7. scores, head-pair batched: forward exp + mask-mult,
            #         reverse exp into u ----
            p_all = acts.tile([128, H, SN], F32)
            pm = acts.tile([128, H, SN], F32)
            u_all = acts.tile([128, H, SN], BF)
            def hsl(t, h):
                return t[:, h, :]

            for h0 in range(0, H, 2):
                # forward pair: scores[i, j] for heads h0, h0+1
                ps_f = psB.tile([128, 258], F32, tag="psB")
                for g in range(2):
                    qs = hsl(qT, h0 + g)
                    ks = hsl(kT, h0 + g)
                    nc.tensor.matmul(
                        ps_f[:, g * SN : (g + 1) * SN], qs[:, 0:S], ks,
                        start=True, stop=True,
                    )
                nc.scalar.activation(
                    p_all[:, h0 : h0 + 2, :].rearrange("p g j -> p (g j)"),
                    ps_f[:, 0 : 2 * SN],
                    Act.Exp,
                )
                nc.vector.tensor_tensor(
                    pm[:, h0 : h0 + 2, :], p_all[:, h0 : h0 + 2, :],
                    m8[:, h0 : h0 + 2, :], Alu.mult,
                )
                # reverse pair: u[j', i] = exp(scores[i, j'+1])
                ps_r = psB.tile([128, 258], F32, tag="psB")
                for g in range(2):
                    qs = hsl(qT, h0 + g)
                    ks = hsl(kT, h0 + g)
                    nc.tensor.matmul(
                        ps_r[:, g * SN : (g + 1) * SN], ks[:, 1:SN], qs,
                        start=True, stop=True,
                    )
                nc.scalar.activation(
                    u_all[:, h0 : h0 + 2, :].rearrange("p g j -> p (g j)"),
                    ps_r[:, 0 : 2 * SN],
                    Act.Exp,
                )
            # mask u (self-attention) with bf16 multiply on DVE
            nc.vector.tensor_tensor(u_all, u_all, mu, Alu.mult)

            # ---- 8. softmax normalization (rows 0..127, all heads) ----
            a_all_scratch = tmp.tile([128, 4, SN], F32, tag="zscr")
            z = tmp.tile([128, H], F32, tag="z")
            nc.vector.reduce_sum(
                z[:, 0:4], pm[:, 0:4, :], axis=mybir.AxisListType.X
            )
            for h in range(4, H):
                nc.scalar.activation(
                    a_all_scratch[:, h - 4, :], pm[:, h, :], Act.Copy,
                    accum_out=z[:, h : h + 1],
                )
            rz = tmp.tile([128, H], F32, tag="rz")
            nc.vector.reciprocal(rz, z)
            a_all = acts.tile([128, H, SN], F32)
            for h in range(H):
                if h % 2 == 0:
                    nc.scalar.activation(
                        a_all[:, h, :], pm[:, h, :], Act.Copy,
                        scale=rz[:, h : h + 1],
                    )
                else:
                    nc.vector.tensor_scalar_mul(
                        a_all[:, h, :], pm[:, h, :], rz[:, h : h + 1]
                    )
            nc.sync.dma_start(attn_d[0:S, :, :], a_all)

            # ---- 9. attention row i=128 from columns of u (one matmul) ----
            ps_t8 = psC.tile([8, 512], F32, tag="ps1t")
            ucol = u_all[:, :, S : S + 1].rearrange("p h u -> p (h u)")
            nc.tensor.matmul(ps_t8[:, 0:S], ucol, ident_bf, start=True, stop=True)
            t8 = tmp.tile([8, S], F32, tag="t8")
            nc.vector.tensor_copy(t8, ps_t8[:, 0:S])
            z1p = tmp.tile([8, 1], F32, tag="z1")
            nc.vector.reduce_sum(z1p, t8, axis=mybir.AxisListType.X)
            rz1p = tmp.tile([8, 1], F32, tag="rz1")
            nc.vector.reciprocal(rz1p, z1p)
            a1w = tmp.tile([8, SN], F32, tag="a1")
            nc.vector.memset(a1w[:, 0:1], 0.0)
            nc.vector.tensor_scalar_mul(a1w[:, 1:SN], t8, rz1p)
            nc.gpsimd.dma_start(attn_d[S, :, :], a1w)
            # rz1 as a free-major row for ctx row-128 normalization
            rz1_bf = tmp.tile([8, 1], BF, tag="rz1b")
            nc.vector.tensor_copy(rz1_bf, rz1p)
            ps_rzr = psC.tile([8, 512], F32, tag="ps1t")
            nc.tensor.matmul(
                ps_rzr[0:1, 0:8], rz1_bf, ident_bf[0:8, 0:8], start=True, stop=True
            )
            rz1 = tmp.tile([1, H], F32, tag="rz1r")
            nc.vector.tensor_copy(rz1, ps_rzr[0:1, 0:8])

            # ---- 10. ctx = attn @ v  (via unnormalized u, then scale) ----
            ps_c0 = psA.tile([128, D], F32, tag="psA")
            ps_c1 = psC.tile([1, 512], F32, tag="psc1")
            for h in range(H):
                hs = slice(h * HD, (h + 1) * HD)
                nc.tensor.matmul(
                    ps_c0[:, hs], u_all[:, h, 0:S], v_bf[:, hs],
                    start=True, stop=True,
                )
                nc.tensor.matmul(
                    ps_c1[0:1, hs], u_all[:, h, S:SN], v_bf[:, hs],
                    start=True, stop=True,
                )
            ctx0 = acts.tile([128, D], BF)
            nc.vector.tensor_tensor(
                ctx0.rearrange("p (h e) -> p h e", h=H),
                ps_c0.rearrange("p (h e) -> p h e", h=H),
                rz[:, :, None].to_broadcast([128, H, HD]),
                Alu.mult,
            )
            ctx1 = acts.tile([1, D], BF)
            nc.vector.tensor_tensor(
                ctx1.rearrange("p (h e) -> p h e", h=H),
                ps_c1[0:1, 0:D].rearrange("p (h e) -> p h e", h=H),
                rz1[:, :, None].to_broadcast([1, H, HD]),
                Alu.mult,
            )

            # ---- 11. ctxT via PE transposes ----
            ctxT = acts.tile([128, 2, SN], BF)
            for c in range(2):
                cs = slice(c * 128, (c + 1) * 128)
                ps_t0 = psB.tile([128, 258], BF, tag="psB")
                nc.tensor.transpose(ps_t0[:, 0:128], ctx0[:, cs], ident_bf)
                if c == 0:
                    nc.vector.tensor_copy(ctxT[:, c, 0:S], ps_t0[:, 0:128])
                else:
                    nc.scalar.activation(ctxT[:, c, 0:S], ps_t0[:, 0:128], Act.Copy)
                ps_t1 = psB.tile([128, 258], F32, tag="psB")
                nc.tensor.matmul(
                    ps_t1[0:128, 0:1], ctx1[0:1, cs], ones_bf[0:1, 0:1],
                    start=True, stop=True,
                )
                if c == 0:
                    nc.vector.tensor_copy(ctxT[:, c, S:SN], ps_t1[0:128, 0:1])
                else:
                    nc.scalar.activation(
                        ctxT[:, c, S:SN], ps_t1[0:128, 0:1], Act.Copy
                    )

            # ---- 12. outT = (ctx @ Wo + bo)^T  (feature-major [d', i]) ----
            o_sb = tmp.tile([128, 2, SN], F32, tag="osb")
            for c in range(2):
                ps_o = psB.tile([128, 258], F32, tag="psB")
                for k in range(2):
                    nc.tensor.matmul(
                        ps_o[:, 0:SN],
                        wsb2[:, O_WO - 6 + k, c * 128 : (c + 1) * 128],
                        ctxT[:, k, :],
                        start=(k == 0), stop=(k == 1),
                    )
                nc.vector.tensor_scalar_add(
                    o_sb[:, c, :], ps_o[:, 0:SN], bcols[:, R_BO, c : c + 1]
                )
            nc.sync.dma_start(
                outT_d.rearrange("(c p) i -> p c i", p=128), o_sb
            )

    if not nc.is_finalized():
        nc.finalize()
    _NC_CACHE = nc
    return nc


def kernel(desc_embeddings, name_embeddings, value_embeddings, cls_token, params):
    name = np.asarray(name_embeddings, np.float32)
    value = np.asarray(value_embeddings, np.float32)
    cls = np.asarray(cls_token, np.float32).reshape(1, D)
    p = {k: np.asarray(v, np.float32) for k, v in params.items()}

    isq = 1.0 / np.sqrt(np.float32(HD))
    w1 = np.ascontiguousarray(
        np.concatenate([p["sfW1"], p["sfW2"]], axis=0)
        .reshape(6, 128, D).transpose(1, 0, 2)
    ).astype(BF16)  # [128, 6, 256]
    w2 = np.ascontiguousarray(
        np.concatenate([p["Wq"] * isq, p["Wk"], p["Wv"], p["Wo"]], axis=0)
        .reshape(8, 128, D).transpose(1, 0, 2)
    ).astype(BF16)  # [128, 8, 256]
    brows = np.stack(
        [p["sfb1"], p["sfb2"], p["bq"] * isq, p["bk"], p["bv"], p["bo"]]
    ).astype(BF16)  # [6,256]
    gb = np.stack([p["sfg"], p["sfbeta"]]).astype(np.float32)  # [2,256]
    brows_f = np.stack(
        [p["sfb1"], p["sfb2"], p["bq"] * isq, p["bk"], p["bv"], p["bo"]]
    ).astype(np.float32)
    bcols = np.ascontiguousarray(
        brows_f.reshape(6, 2, 128).transpose(2, 0, 1)
    )  # [128, 6, 2]
    clsT = np.ascontiguousarray(cls.reshape(2, 128, 1).transpose(1, 0, 2)).astype(BF16)  # [128,2,1]

    nc = _build_nc()
    in_maps = []
    for b in range(B):
        nvT = np.ascontiguousarray(
            np.concatenate([name[b].T, value[b].T], axis=0)
            .reshape(4, 128, S).transpose(1, 0, 2)
        ).astype(BF16)  # [128, 4, 128]
        in_maps.append(
            {
                "nvT": nvT, "clsT": clsT, "W1": w1, "W2": w2,
                "brows": brows, "gb": gb, "bcols": bcols,
            }
        )
    res = run_bass_kernel_spmd(nc, in_maps, core_ids=list(range(NCORES)))

    out = np.empty((B, SN, D), np.float32)
    attn = np.empty((B, H, SN, SN), np.float32)
    for b in range(B):
        out[b] = np.ascontiguousarray(res.results[b]["outT"].T)
        attn[b] = res.results[b]["attn"].transpose(1, 0, 2)
    return out, attn
